# revision 1
# baseline (speedup 1.0000x reference)
"""Trainium2 Bass kernel for nn_Block_70093866270826.

Sharding: token-data-parallel across 8 cores (the entire block is per-token
math: rotary, LN, per-token windowed attention, MLP — no cross-token mixing),
so each core processes 256 of the 2048 tokens with full weights. No
collectives.

Layouts: feature-major [feat_part, tok_free] for the matmul chain (PE,
fp32r = 1 cyc/row), token-major [tok_part, (d,v)_free] for the attention
island (DVE/ACT/GPSIMD elementwise with step-0 broadcast APs). PE transposes
at the boundaries.
"""
import sys

sys.path.insert(0, "/opt/trn_rl_repo")

import ml_dtypes
import numpy as np

import concourse.bass as bass
import concourse.tile as tile
from concourse import bacc, mybir
from concourse.bass import AP
from concourse.bass_utils import run_bass_kernel_spmd
from concourse.masks import make_identity

F32 = mybir.dt.float32
F32R = mybir.dt.float32r
BF16 = mybir.dt.bfloat16
ALU = mybir.AluOpType
ACTF = mybir.ActivationFunctionType
AXX = mybir.AxisListType.X

B, T, E, H, W = 2, 1024, 1024, 8, 31
D = 2 * W + 1            # 63
DD = D * D               # 3969
HD = H * D               # 504
HDP = 512
E4 = 4 * E
NCORES = 8
TLOC = (B * T) // NCORES  # 256
NT = TLOC // 128          # 2
PI = float(np.pi)
TWO_PI = float(2 * np.pi)
EPS = 1e-5


def _bcast_mid(ap_2d: AP, n: int) -> AP:
    """[P, m] -> [P, n(bcast), m] (step-0 middle dim)."""
    return AP(tensor=ap_2d.tensor, offset=ap_2d.offset,
              ap=[list(ap_2d.ap[0]), [0, n], list(ap_2d.ap[1])])


def emit(nc, tc, io, ctx, knobs):
    iters = knobs.get("iters", 0)
    upto0 = knobs.get("upto", "full")
    if iters:
        ctx.enter_context(tc.For_i(0, iters, 1))
    consts = ctx.enter_context(tc.tile_pool(name="consts", bufs=1))
    acts = ctx.enter_context(tc.tile_pool(name="acts", bufs=1))
    attn_pool = ctx.enter_context(tc.tile_pool(name="attn", bufs=knobs.get("attn_bufs", 2)))
    wq = ctx.enter_context(tc.tile_pool(name="wq", bufs=3))
    wf = ctx.enter_context(tc.tile_pool(name="wf", bufs=1))
    wc = ctx.enter_context(tc.tile_pool(name="wc", bufs=2))
    m1p = ctx.enter_context(tc.tile_pool(name="m1p", bufs=1))
    tmp = ctx.enter_context(tc.tile_pool(name="tmp", bufs=2))
    tmps = ctx.enter_context(tc.tile_pool(name="tmps", bufs=3))
    # PSUM: 4 pools x 2 banks = 8 banks exactly; every tile uses its pool tag.
    ps1 = ctx.enter_context(tc.tile_pool(name="ps1", bufs=2, space="PSUM"))
    ps2 = ctx.enter_context(tc.tile_pool(name="ps2", bufs=2, space="PSUM"))
    ps3 = ctx.enter_context(tc.tile_pool(name="ps3", bufs=2, space="PSUM"))
    cpp = ctx.enter_context(tc.tile_pool(name="cpp", bufs=2, space="PSUM"))

    # ---------------- constants ----------------
    ident = consts.tile([128, 128], F32, name='ident')
    make_identity(nc, ident[:])

    if not knobs.get("attn_bf16", True):
        ctile = consts.tile([128, DD], F32, name='ctile')
        nc.sync.dma_start(ctile[:], io["crow"].partition_broadcast(128))
        ctile3 = ctile[:].rearrange("p (d v) -> p d v", d=D)
    else:
        ctile3 = None

    qkvb_r = []
    qkvb_src = io["qkvb"].rearrange("(o f) -> o f", o=1)
    for c in range(3):
        t = consts.tile([1, HD], F32, tag=f"qkvbr{c}", name=f"qkvbr{c}")
        nc.sync.dma_start(t[:], qkvb_src[:, c * HD:(c + 1) * HD])
        qkvb_r.append(t)

    def ppart_vec(name, dram, n):
        tiles = []
        src = dram.rearrange("(n p o) -> n p o", p=128, o=1)
        for i in range(n // 128):
            t = consts.tile([128, 1], F32, tag=f"{name}{i}", name=f"{name}{i}")
            nc.sync.dma_start(t[:], src[i])
            tiles.append(t)
        return tiles

    def row_vec(name, dram, n):
        """[n] dram -> list of [1,128] row tiles."""
        tiles = []
        src = dram.rearrange("(o f) -> o f", o=1)
        for i in range(n // 128):
            t = consts.tile([1, 128], F32, tag=f"{name}{i}", name=f"{name}{i}")
            nc.sync.dma_start(t[:], src[:, i * 128:(i + 1) * 128])
            tiles.append(t)
        return tiles

    invfreq_t = ppart_vec("invf", io["invfreq"], 512)
    projb_t = ppart_vec("projb", io["projb"], E)
    fcb_t = ppart_vec("fcb", io["fcb"], E4)
    cprojb_t = ppart_vec("cprojb", io["cprojb"], E)
    ln1w_r = row_vec("ln1w", io["ln1w"], E)
    ln1b_r = row_vec("ln1b", io["ln1b"], E)
    ln2w_r = row_vec("ln2w", io["ln2w"], E)
    ln2b_r = row_vec("ln2b", io["ln2b"], E)

    def sconst(val, name):
        t = consts.tile([128, 1], F32, tag=name)
        nc.vector.memset(t[:], float(val))
        return t

    c_pi = sconst(PI, "c_pi")
    c_negpi = sconst(-PI, "c_negpi")
    c_halfpi = sconst(PI / 2, "c_halfpi")
    c_neg3halfpi = sconst(-1.5 * PI, "c_neg3halfpi")
    c_n2pi = sconst(-TWO_PI, "c_n2pi")
    c_p2pi = sconst(TWO_PI, "c_p2pi")
    c_eps = sconst(EPS, "c_eps")
    ones_col = sconst(1.0, "ones_col")              # [128, 1]
    ones_256 = consts.tile([1, TLOC], F32, tag="ones_256", name="ones_256")
    nc.vector.memset(ones_256[:], 1.0)

    if upto0 == "noop":
        for m in range(NT):
            z = tmp.tile([128, E], F32, tag="znoop", name="znoop")
            nc.vector.memset(z[:], 0.0)
            nc.sync.dma_start(io["y"].rearrange("(n p) f -> n p f", p=128)[m], z[:])
        return

    # ---------------- load + transpose x ----------------
    xT = [acts.tile([128, TLOC], F32, tag=f"xaT{i}", name=f"xT{i}") for i in range(4)]
    for m in range(NT):
        xtile = tmp.tile([128, 512], F32, tag="xin", name="xin", bufs=1)
        nc.sync.dma_start(xtile[:], io["x"].rearrange("(n p) f -> n p f", p=128)[m])
        for i in range(4):
            ps = ps3.tile([128, 512], F32, tag="ps3", name="ps3")
            nc.tensor.transpose(ps[:, :128], xtile[:, i * 128:(i + 1) * 128], ident[:])
            nc.scalar.copy(xT[i][:, m * 128:(m + 1) * 128], ps[:, :128])

    # ---------------- rotary ----------------
    xrT = [acts.tile([128, TLOC], F32, tag=f"xrT{i}", name=f"xrT{i}") for i in range(8)]
    for i in range(4):
        ang = tmp.tile([128, TLOC], F32, tag="ang", name="ang")
        nc.vector.tensor_scalar(ang[:], xT[i][:], invfreq_t[i][:], None, ALU.mult)
        m1 = tmp.tile([128, TLOC], F32, tag="m1", name="m1")
        m2 = tmp.tile([128, TLOC], F32, tag="m2", name="m2")
        r = tmp.tile([128, TLOC], F32, tag="r", name="r")
        nc.vector.tensor_scalar(m1[:], ang[:], c_pi[:], None, ALU.is_gt)
        nc.vector.tensor_scalar(m2[:], ang[:], c_negpi[:], None, ALU.is_lt)
        nc.vector.scalar_tensor_tensor(r[:], m1[:], c_n2pi[:], ang[:], ALU.mult, ALU.add)
        nc.vector.scalar_tensor_tensor(r[:], m2[:], c_p2pi[:], r[:], ALU.mult, ALU.add)
        nc.scalar.activation(xrT[i][:], r[:], ACTF.Sin)
        nc.vector.tensor_scalar(m1[:], ang[:], c_halfpi[:], None, ALU.is_gt)
        nc.vector.tensor_scalar(m2[:], ang[:], c_neg3halfpi[:], None, ALU.is_lt)
        nc.vector.scalar_tensor_tensor(r[:], m1[:], c_n2pi[:], ang[:], ALU.mult, ALU.add)
        nc.vector.scalar_tensor_tensor(r[:], m2[:], c_p2pi[:], r[:], ALU.mult, ALU.add)
        nc.scalar.activation(xrT[4 + i][:], r[:], ACTF.Sin, bias=c_halfpi[:])

    upto = knobs.get("upto", "full")

    def finish_featmajor(tiles8):
        for e in range(8):
            src_t = tiles8[e]
            sap = src_t[:] if src_t.dtype == F32 else src_t[:].bitcast(F32)
            for m in range(NT):
                ps = ps3.tile([128, 512], F32, tag="ps3", name="ps3f")
                nc.tensor.transpose(ps[:, :128], sap[:, m * 128:(m + 1) * 128], ident[:])
                ysb = tmp.tile([128, 128], F32, tag="ysb", name="ysbf")
                nc.scalar.copy(ysb[:], ps[:, :128])
                nc.sync.dma_start(
                    io["y"].rearrange("(n p) f -> n p f", p=128)[m, :, e * 128:(e + 1) * 128],
                    ysb[:])

    def finish_tokmajor(tiles_m, width):
        for m in range(NT):
            nc.sync.dma_start(
                io["y"].rearrange("(n p) f -> n p f", p=128)[m, :, :width],
                tiles_m[m][:, :width])
            if width < E:
                z = tmp.tile([128, E - width], F32, tag="zpad", name="zpad")
                nc.vector.memset(z[:], 0.0)
                nc.sync.dma_start(
                    io["y"].rearrange("(n p) f -> n p f", p=128)[m, :, width:],
                    z[:])

    if upto == "rotary":
        finish_featmajor(xrT)
        return

    # ---------------- layernorm helper (feat-major over 8 tiles) ----------------
    def layernorm(src_tiles, w_rows, b_rows, out_tag, out_dt=F32R):
        sum_ps = ps1.tile([128, 512], F32, tag="ps1", name="ps1")
        sq_ps = ps2.tile([128, 512], F32, tag="ps2", name="ps2")
        for i in range(8):
            nc.tensor.matmul(sum_ps[:1, :TLOC], ones_col[:], src_tiles[i][:],
                             start=(i == 0), stop=(i == 7))
        for i in range(8):
            sq = tmp.tile([128, TLOC], F32, tag="lnsq", name="lnsq")
            nc.scalar.activation(sq[:], src_tiles[i][:], ACTF.Square)
            nc.tensor.matmul(sq_ps[:1, :TLOC], ones_col[:], sq[:],
                             start=(i == 0), stop=(i == 7))
        row = tmps.tile([1, 4 * TLOC], F32, tag="lnrow", name="lnrow", bufs=1)
        mu = row[:, 0:TLOC]
        var = row[:, TLOC:2 * TLOC]
        rstd = row[:, 2 * TLOC:3 * TLOC]
        nrm = row[:, 3 * TLOC:4 * TLOC]
        nc.scalar.mul(mu, sum_ps[:1, :TLOC], 1.0 / E)
        nc.vector.tensor_tensor(nrm, mu, mu, ALU.mult)  # nrm as musq scratch
        nc.vector.scalar_tensor_tensor(var, sq_ps[:1, :TLOC], 1.0 / E, nrm,
                                       ALU.mult, ALU.subtract)
        nc.vector.tensor_scalar(var, var, c_eps[:1, :], None, ALU.add)
        nc.scalar.activation(var, var, ACTF.Ln)
        nc.scalar.activation(rstd, var, ACTF.Exp, scale=-0.5)
        nc.vector.tensor_tensor(nrm, mu, rstd, ALU.mult)
        nc.scalar.mul(nrm, nrm, -1.0)
        outs = []
        for i in range(8):
            a_ps = ps1.tile([128, 512], F32, tag="ps1", name="ps1")
            b_ps = ps2.tile([128, 512], F32, tag="ps2", name="ps2")
            nc.tensor.matmul(a_ps[:, :TLOC], w_rows[i][:], rstd, start=True, stop=True)
            nc.tensor.matmul(b_ps[:, :TLOC], w_rows[i][:], nrm, start=True, stop=False)
            nc.tensor.matmul(b_ps[:, :TLOC], b_rows[i][:], ones_256[:], start=False, stop=True)
            o = acts.tile([128, TLOC], out_dt, tag=f"{out_tag}{i}", name=f"{out_tag}{i}")
            t1 = tmp.tile([128, TLOC], F32, tag="lnt1", name="lnt1")
            nc.vector.tensor_tensor(t1[:], src_tiles[i][:], a_ps[:, :TLOC], ALU.mult)
            nc.vector.tensor_tensor(o[:], t1[:], b_ps[:, :TLOC], ALU.add)
            outs.append(o)
        return outs

    qkv_bf16 = knobs.get("qkv_bf16", True)
    hT = layernorm(xrT, ln1w_r, ln1b_r, "lnout", out_dt=(BF16 if qkv_bf16 else F32R))
    if upto == "ln1":
        finish_featmajor(hT)
        return

    # ---------------- qkv (token-major out) ----------------
    qkv_sb = [acts.tile([128, 3 * HD], F32, tag=f"qkv{m}", name=f"qkv{m}") for m in range(NT)]
    qkvw_src = io["qkvw_t"].rearrange("(n p) f -> n p f", p=128)
    qkvw_b_src = io["qkvw_tb"].rearrange("(n p) f -> n p f", p=128)
    for c in range(3):
        pss = [ps1.tile([128, 512], F32, tag="ps1", name="ps1") for _ in range(NT)]
        for k in range(8):
            wt = wq.tile([128, HD], BF16 if qkv_bf16 else F32R, tag="qkvw", name="qkvw")
            nc.sync.dma_start(wt[:], (qkvw_b_src if qkv_bf16 else qkvw_src)[k, :, c * HD:(c + 1) * HD])
            for m in range(NT):
                nc.tensor.matmul(pss[m][:, :HD], hT[k][:, m * 128:(m + 1) * 128],
                                 wt[:], start=(k == 0), stop=False)
        for m in range(NT):
            nc.tensor.matmul(pss[m][:, :HD], ones_256[:, :128], qkvb_r[c][:],
                             start=False, stop=True)
            nc.scalar.copy(qkv_sb[m][:, c * HD:(c + 1) * HD], pss[m][:, :HD])

    if upto == "qkv":
        finish_tokmajor(qkv_sb, E)
        return

    # ---------------- attention ----------------
    attn_eng = knobs.get("attn_eng", "ddd")   # engines for (s0, F, P2): d=DVE g=GPSIMD
    attn_bf16 = knobs.get("attn_bf16", True)
    eng = {"d": nc.vector, "g": nc.gpsimd}
    SDT = BF16 if attn_bf16 else F32
    if attn_bf16:
        ctile_b = consts.tile([128, DD], BF16, tag="ctile_b", name="ctile_b")
        nc.sync.dma_start(ctile_b[:], io["crow_b"].partition_broadcast(128))
        ctile3_x = ctile_b[:].rearrange("p (d v) -> p d v", d=D)
    else:
        ctile3_x = ctile3

    s0_bf16 = knobs.get("s0_bf16", False)
    S0DT = BF16 if s0_bf16 else F32
    OUT = [acts.tile([128, HDP], F32, tag=f"attnout{m}", name=f"attnout{m}")
           for m in range(NT)]
    for m in range(NT):
        nc.vector.memset(OUT[m][:, HD:HDP], 0.0)
    for bi in range(NT * H):
        m, h = bi // H, bi % H
        out_t = OUT[m]
        if True:
            q = qkv_sb[m][:, h * D:(h + 1) * D]
            kk = qkv_sb[m][:, HD + h * D: HD + (h + 1) * D]
            vv = qkv_sb[m][:, 2 * HD + h * D: 2 * HD + (h + 1) * D]
            st = attn_pool.tile([128, D, D], S0DT, tag="score", name="score", bufs=2)
            eng[attn_eng[0]].tensor_tensor(st[:], q.broadcast_to([128, D, D]),
                                           _bcast_mid(kk, D), ALU.mult)
            ex = attn_pool.tile([128, D, D], SDT, tag="escore", name="escore")                 if attn_bf16 else st
            nc.scalar.activation(ex[:], st[:], ACTF.Exp)
            eng[attn_eng[1]].tensor_tensor(ex[:], ex[:], ctile3_x, ALU.mult)
            g = tmps.tile([128, D], F32, tag="g", name="g")
            g_in = ex[:] if knobs.get("fake_g") else ex[:].rearrange("p d v -> p v d")
            nc.vector.tensor_reduce(g[:], g_in, axis=AXX, op=ALU.add)
            u = tmps.tile([128, D], SDT, tag="u", name="u")
            nc.vector.reciprocal(g[:], g[:])
            nc.vector.tensor_tensor(u[:], g[:], vv, ALU.mult)
            eng[attn_eng[2]].tensor_tensor(ex[:], ex[:], _bcast_mid(u[:], D), ALU.mult)
            nc.vector.tensor_reduce(out_t[:, h * D:(h + 1) * D], ex[:], axis=AXX, op=ALU.add)

    if upto == "attn":
        finish_tokmajor(OUT, HDP)
        return

    # ---------------- transpose OUT -> outT (fp32r) ----------------
    outT = [acts.tile([128, TLOC], BF16, tag=f"lnout{i}", name=f"outT{i}") for i in range(4)]
    for m in range(NT):
        for i in range(4):
            ps = ps3.tile([128, 512], F32, tag="ps3", name="ps3")
            nc.tensor.transpose(ps[:, :128], OUT[m][:, i * 128:(i + 1) * 128], ident[:])
            nc.vector.tensor_copy(outT[i][:, m * 128:(m + 1) * 128], ps[:, :128])

    # ---------------- proj + residual ----------------
    pw = []
    pw_src = io["pw_tb"].rearrange("(n p) f -> n p f", p=128)
    for k in range(4):
        wt = wq.tile([128, E], BF16, tag=f"pw{k}", name=f"pw{k}", bufs=1)
        nc.sync.dma_start(wt[:], pw_src[k])
        pw.append(wt)
    xaT = []
    for i in range(8):
        ps = ps1.tile([128, 512], F32, tag="ps1", name="ps1")
        for k in range(4):
            nc.tensor.matmul(ps[:, :TLOC], pw[k][:, i * 128:(i + 1) * 128], outT[k][:],
                             start=(k == 0), stop=(k == 3))
        o = acts.tile([128, TLOC], F32, tag=f"xaT{i}", name=f"xaT{i}")
        nc.vector.scalar_tensor_tensor(o[:], ps[:, :TLOC], projb_t[i][:], xrT[i][:],
                                       ALU.add, ALU.add)
        xaT.append(o)

    if upto == "proj":
        finish_featmajor(xaT)
        return

    # ---------------- LN2 ----------------
    h2T = layernorm(xaT, ln2w_r, ln2b_r, "lnout", out_dt=BF16)  # reuses lnout slots

    # ---------------- fc + gelu -> m1g (bf16), then cproj ----------------
    m1g = [m1p.tile([128, TLOC], BF16, tag=f"m1g{j}", name=f"m1g{j}") for j in range(32)]
    fw_g = io["fw_t"].rearrange("(k p) (g f) -> p k g f", p=128, f=256)  # [128,8,16,256]
    for jg in range(16):          # groups of 2 j-tiles
        fwg = wf.tile([128, 8, 256], BF16, tag="fwg", name="fwg", bufs=4)
        nc.sync.dma_start(fwg[:], fw_g[:, :, jg, :])
        for jj in range(2):
            j = jg * 2 + jj
            fps = ps2.tile([128, 512], F32, tag="ps2", name="ps2")
            for k in range(8):
                nc.tensor.matmul(fps[:, :TLOC], fwg[:, k, jj * 128:(jj + 1) * 128],
                                 h2T[k][:], start=(k == 0), stop=(k == 7))
            gelu_f = ACTF.Tanh if knobs.get("sim_tanh") else ACTF.Gelu
            nc.scalar.activation(m1g[j][:], fps[:, :TLOC], gelu_f, bias=fcb_t[j][:])
    # cproj: e-outer, contract over 32 j-tiles
    cw_src = io["cw_te"].rearrange("(e p) f -> e p f", p=128)    # [8,128,4096]
    for e in range(8):
        cps = cpp.tile([128, 512], F32, tag="cpp", name="cpp")
        for half in range(2):
            cwt = wc.tile([128, E4 // 2], BF16, tag="cwt", name="cwt")
            nc.sync.dma_start(cwt[:], cw_src[e, :, half * 2048:(half + 1) * 2048])
            for jj in range(16):
                j = half * 16 + jj
                nc.tensor.matmul(cps[:, :TLOC], cwt[:, jj * 128:(jj + 1) * 128],
                                 m1g[j][:], start=(j == 0), stop=(j == 31))
        yT = tmp.tile([128, TLOC], F32, tag="yT", name="yT")
        nc.vector.scalar_tensor_tensor(yT[:], cps[:, :TLOC], cprojb_t[e][:], xaT[e][:],
                                       ALU.add, ALU.add)
        for m in range(NT):
            ps = ps3.tile([128, 512], F32, tag="ps3", name="ps3")
            nc.tensor.transpose(ps[:, :128], yT[:, m * 128:(m + 1) * 128], ident[:])
            ysb = tmp.tile([128, 128], F32, tag="ysb", name="ysb")
            nc.scalar.copy(ysb[:], ps[:, :128])
            nc.sync.dma_start(
                io["y"].rearrange("(n p) f -> n p f", p=128)[m, :, e * 128:(e + 1) * 128],
                ysb[:])


def build(knobs=None):
    from contextlib import ExitStack
    knobs = knobs or {}
    nc = bacc.Bacc("TRN2", target_bir_lowering=False, debug=False)
    io = {}

    def din(name, shape, dt=F32):
        io[name] = nc.dram_tensor(name, shape, dt, kind="ExternalInput").ap()

    din("x", [TLOC, 512])
    din("qkvw_t", [E, 3 * HD], F32R)
    din("qkvw_tb", [E, 3 * HD], BF16)
    din("pw_t", [HDP, E], F32R)
    din("pw_tb", [HDP, E], BF16)
    din("fw_t", [E, E4], BF16)
    din("cw_te", [E, E4], BF16)     # per-e k-major packing, see host_prep
    din("crow", [DD])
    din("crow_b", [DD], BF16)
    din("invfreq", [512])
    din("ln1w", [E]); din("ln1b", [E])
    din("ln2w", [E]); din("ln2b", [E])
    din("qkvb", [3 * HD])
    din("projb", [E]); din("fcb", [E4]); din("cprojb", [E])
    io["y"] = nc.dram_tensor("y", [TLOC, E], F32, kind="ExternalOutput").ap()

    with tile.TileContext(nc) as tc:
        with ExitStack() as ctx:
            emit(nc, tc, io, ctx, knobs)
    nc.compile()
    return nc


def host_prep(inputs):
    x = np.asarray(inputs["x"], np.float32).reshape(B * T, E // 2)
    qkv_w = np.asarray(inputs["qkv_w"], np.float32)
    rel_pos = np.asarray(inputs["rel_pos"], np.float32)
    proj_w = np.asarray(inputs["proj_w"], np.float32)
    fc_w = np.asarray(inputs["fc_w"], np.float32)
    cproj_w = np.asarray(inputs["cproj_w"], np.float32)

    inv_freq = (1.0 / 10000.0 ** (np.arange(0, E, 2, dtype=np.float32) / E)).astype(np.float32)
    perm = np.arange(-W, W + 1) % D
    crow = np.exp(rel_pos[perm]).astype(np.float32).reshape(-1)

    pw_t = np.zeros((HDP, E), np.float32)
    pw_t[:HD] = proj_w.T

    # cw_te[e]: [4096, 128] column-block e of cproj_w.T, repacked so SBUF tile
    # [128, 4096] holds k-tile j at cols j*128:(j+1)*128
    cw_t = cproj_w.T.astype(ml_dtypes.bfloat16)          # [4096, 1024]
    cw_te = np.empty((E, E4), ml_dtypes.bfloat16)
    for e in range(8):
        blk = cw_t[:, e * 128:(e + 1) * 128]             # [4096, 128]
        cw_te[e * 128:(e + 1) * 128] = (
            blk.reshape(32, 128, 128).transpose(1, 0, 2).reshape(128, E4))

    common = {
        "qkvw_t": np.ascontiguousarray(qkv_w.T),
        "qkvw_tb": np.ascontiguousarray(qkv_w.T.astype(ml_dtypes.bfloat16)),
        "pw_t": pw_t,
        "pw_tb": pw_t.astype(ml_dtypes.bfloat16),
        "fw_t": np.ascontiguousarray(fc_w.T.astype(ml_dtypes.bfloat16)),
        "cw_te": cw_te,
        "crow": crow,
        "crow_b": crow.astype(ml_dtypes.bfloat16),
        "invfreq": inv_freq,
        "ln1w": np.asarray(inputs["ln1_w"], np.float32),
        "ln1b": np.asarray(inputs["ln1_b"], np.float32),
        "ln2w": np.asarray(inputs["ln2_w"], np.float32),
        "ln2b": np.asarray(inputs["ln2_b"], np.float32),
        "qkvb": np.asarray(inputs["qkv_b"], np.float32),
        "projb": np.asarray(inputs["proj_b"], np.float32),
        "fcb": np.asarray(inputs["fc_b"], np.float32),
        "cprojb": np.asarray(inputs["cproj_b"], np.float32),
    }
    in_maps = []
    for c in range(NCORES):
        m = dict(common)
        m["x"] = np.ascontiguousarray(x[c * TLOC:(c + 1) * TLOC])
        in_maps.append(m)
    return in_maps


def kernel(**inputs):
    nc = build()
    in_maps = host_prep(inputs)
    res = run_bass_kernel_spmd(nc, in_maps, list(range(NCORES))).results
    y = np.concatenate([res[c]["y"] for c in range(NCORES)], axis=0)
    return y.reshape(B, T, E)



# revision 3
# speedup vs baseline: 3.0705x; 3.0705x over previous
"""Trainium2 Bass kernel for nn_Block_70093866270826.

Sharding: token-data-parallel across 8 cores (the entire block is per-token
math: rotary, LN, per-token windowed attention, MLP — no cross-token mixing),
so each core processes 256 of the 2048 tokens with full weights. No
collectives.

Attention: the per-token softmax over exp(q_d*k_v + B_dv) is evaluated via a
truncated-exp rank decomposition: exp(q*k) = sum_n (q^n/n!) k^n, so both the
softmax denominator g[t,v] = sum_d exp(.)e^B and the value application
out[t,d] = sum_v exp(.)e^B u[t,v] become PE matmuls against the constant
(e^B / n!) matrices, with Horner/ascending accumulation over n on the DVE in
bf16 (2x mode). Heads are packed two per 126-partition tile with
block-diagonal weight matrices. Truncation error at N=8 is ~3e-4 relative,
far below the bf16 noise floor.

Layouts: feature-major [feat_part, tok_free] for the matmul chain; the
attention island is feature-major too ([126 = 2*63 head-pair rows,
4 pairs x 256 tokens] tiles), so no transposes are needed between qkv,
attention, and proj.
"""
import math
import sys

sys.path.insert(0, "/opt/trn_rl_repo")

import ml_dtypes
import numpy as np

import concourse.bass as bass
import concourse.tile as tile
from concourse import bacc, mybir
from concourse.bass import AP
from concourse.bass_utils import run_bass_kernel_spmd
from concourse.masks import make_identity

F32 = mybir.dt.float32
BF16 = mybir.dt.bfloat16
ALU = mybir.AluOpType
ACTF = mybir.ActivationFunctionType
AXX = mybir.AxisListType.X

B, T, E, H, W = 2, 1024, 1024, 8, 31
D = 2 * W + 1            # 63
P2 = 2 * D               # 126 partitions = head pair
NPAIR = H // 2           # 4
HD = H * D               # 504
E4 = 4 * E
NCORES = 8
TLOC = (B * T) // NCORES  # 256
NT = TLOC // 128          # 2
FDA = NPAIR * TLOC        # 1024: attention tile free size
NPOLY = 8                 # exp() Taylor degree
PI = float(np.pi)
TWO_PI = float(2 * np.pi)
EPS = 1e-5


def emit(nc, tc, io, ctx, knobs):
    iters = knobs.get("iters", 0)
    upto = knobs.get("upto", "full")
    if iters:
        ctx.enter_context(tc.For_i(0, iters, 1))
    consts = ctx.enter_context(tc.tile_pool(name="consts", bufs=1))
    acts = ctx.enter_context(tc.tile_pool(name="acts", bufs=1))
    wq = ctx.enter_context(tc.tile_pool(name="wq", bufs=3))
    wf = ctx.enter_context(tc.tile_pool(name="wf", bufs=1))
    wc = ctx.enter_context(tc.tile_pool(name="wc", bufs=2))
    m1p = ctx.enter_context(tc.tile_pool(name="m1p", bufs=1))
    tmp = ctx.enter_context(tc.tile_pool(name="tmp", bufs=2))
    tmps = ctx.enter_context(tc.tile_pool(name="tmps", bufs=3))
    ghp = ctx.enter_context(tc.tile_pool(name="ghp", bufs=3))
    # PSUM: psA/psB one bank x2 bufs, psG two banks x2 bufs = 8 banks exactly.
    psA = ctx.enter_context(tc.tile_pool(name="psA", bufs=2, space="PSUM"))
    psB = ctx.enter_context(tc.tile_pool(name="psB", bufs=2, space="PSUM"))
    psG = ctx.enter_context(tc.tile_pool(name="psG", bufs=2, space="PSUM"))

    # ---------------- constants ----------------
    ident = consts.tile([128, 128], F32, name='ident')
    make_identity(nc, ident[:])

    ebg = consts.tile([P2, (NPOLY + 1) * P2], BF16, name='ebg')
    nc.sync.dma_start(ebg[:], io["ebg"])
    ebh = consts.tile([P2, (NPOLY + 1) * P2], BF16, name='ebh')
    nc.sync.dma_start(ebh[:], io["ebh"])

    qkvb_row = consts.tile([1, 3 * HD], F32, name='qkvb_row')
    nc.sync.dma_start(qkvb_row[:], io["qkvb_pk"].rearrange("(o f) -> o f", o=1))

    def ppart_vec(name, dram, n):
        tiles = []
        src = dram.rearrange("(n p o) -> n p o", p=128, o=1)
        for i in range(n // 128):
            t = consts.tile([128, 1], F32, tag=f"{name}{i}", name=f"{name}{i}")
            nc.sync.dma_start(t[:], src[i])
            tiles.append(t)
        return tiles

    def row_vec(name, dram, n):
        tiles = []
        src = dram.rearrange("(o f) -> o f", o=1)
        for i in range(n // 128):
            t = consts.tile([1, 128], F32, tag=f"{name}{i}", name=f"{name}{i}")
            nc.sync.dma_start(t[:], src[:, i * 128:(i + 1) * 128])
            tiles.append(t)
        return tiles

    invfreq_t = ppart_vec("invf", io["invfreq"], 512)
    projb_t = ppart_vec("projb", io["projb"], E)
    fcb_t = ppart_vec("fcb", io["fcb"], E4)
    cprojb_t = ppart_vec("cprojb", io["cprojb"], E)
    ln1w_r = row_vec("ln1w", io["ln1w"], E)
    ln1b_r = row_vec("ln1b", io["ln1b"], E)
    ln2w_r = row_vec("ln2w", io["ln2w"], E)
    ln2b_r = row_vec("ln2b", io["ln2b"], E)

    def sconst(val, name):
        t = consts.tile([128, 1], F32, tag=name)
        nc.vector.memset(t[:], float(val))
        return t

    c_pi = sconst(PI, "c_pi")
    c_negpi = sconst(-PI, "c_negpi")
    c_halfpi = sconst(PI / 2, "c_halfpi")
    c_neg3halfpi = sconst(-1.5 * PI, "c_neg3halfpi")
    c_n2pi = sconst(-TWO_PI, "c_n2pi")
    c_p2pi = sconst(TWO_PI, "c_p2pi")
    c_eps = sconst(EPS, "c_eps")
    ones_col = sconst(1.0, "ones_col")              # [128, 1]
    ones_256 = consts.tile([1, TLOC], F32, tag="ones_256", name="ones_256")
    nc.vector.memset(ones_256[:], 1.0)
    ones_phi = consts.tile([P2, FDA], BF16, tag="ones_phi", name="ones_phi")
    nc.vector.memset(ones_phi[:], 1.0)

    if upto == "noop":
        for m in range(NT):
            z = tmp.tile([128, E], F32, tag="znoop", name="znoop")
            nc.vector.memset(z[:], 0.0)
            nc.sync.dma_start(io["y"].rearrange("(n p) f -> n p f", p=128)[m], z[:])
        return

    # ---------------- load + transpose x ----------------
    xT = [acts.tile([128, TLOC], F32, tag=f"xaT{i}", name=f"xT{i}") for i in range(4)]
    for m in range(NT):
        xtile = tmp.tile([128, 512], F32, tag="xin", name="xin", bufs=1)
        nc.sync.dma_start(xtile[:], io["x"].rearrange("(n p) f -> n p f", p=128)[m])
        for i in range(4):
            ps = psA.tile([128, 512], F32, tag="psA", name="psA")
            nc.tensor.transpose(ps[:, :128], xtile[:, i * 128:(i + 1) * 128], ident[:])
            nc.scalar.copy(xT[i][:, m * 128:(m + 1) * 128], ps[:, :128])

    # ---------------- rotary ----------------
    xrT = [acts.tile([128, TLOC], F32, tag=f"xrT{i}", name=f"xrT{i}") for i in range(8)]
    for i in range(4):
        ang = tmp.tile([128, TLOC], F32, tag="ang", name="ang")
        nc.vector.tensor_scalar(ang[:], xT[i][:], invfreq_t[i][:], None, ALU.mult)
        m1 = tmp.tile([128, TLOC], F32, tag="m1", name="m1")
        m2 = tmp.tile([128, TLOC], F32, tag="m2", name="m2")
        r = tmp.tile([128, TLOC], F32, tag="r", name="r")
        nc.vector.tensor_scalar(m1[:], ang[:], c_pi[:], None, ALU.is_gt)
        nc.vector.tensor_scalar(m2[:], ang[:], c_negpi[:], None, ALU.is_lt)
        nc.vector.scalar_tensor_tensor(r[:], m1[:], c_n2pi[:], ang[:], ALU.mult, ALU.add)
        nc.vector.scalar_tensor_tensor(r[:], m2[:], c_p2pi[:], r[:], ALU.mult, ALU.add)
        nc.scalar.activation(xrT[i][:], r[:], ACTF.Sin)
        nc.vector.tensor_scalar(m1[:], ang[:], c_halfpi[:], None, ALU.is_gt)
        nc.vector.tensor_scalar(m2[:], ang[:], c_neg3halfpi[:], None, ALU.is_lt)
        nc.vector.scalar_tensor_tensor(r[:], m1[:], c_n2pi[:], ang[:], ALU.mult, ALU.add)
        nc.vector.scalar_tensor_tensor(r[:], m2[:], c_p2pi[:], r[:], ALU.mult, ALU.add)
        nc.scalar.activation(xrT[4 + i][:], r[:], ACTF.Sin, bias=c_halfpi[:])

    def finish_featmajor(tiles8):
        for e in range(8):
            src_t = tiles8[e]
            sap = src_t[:] if src_t.dtype == F32 else src_t[:].bitcast(F32)
            for m in range(NT):
                ps = psA.tile([128, 512], F32, tag="psA", name="psAf")
                nc.tensor.transpose(ps[:, :128], sap[:, m * 128:(m + 1) * 128], ident[:])
                ysb = tmp.tile([128, 128], F32, tag="ysb", name="ysbf")
                nc.scalar.copy(ysb[:], ps[:, :128])
                nc.sync.dma_start(
                    io["y"].rearrange("(n p) f -> n p f", p=128)[m, :, e * 128:(e + 1) * 128],
                    ysb[:])

    if upto == "rotary":
        finish_featmajor(xrT)
        return

    # ---------------- layernorm helper (feat-major over 8 tiles) ----------------
    def layernorm(src_tiles, w_rows, b_rows, out_tag, out_dt=BF16):
        sum_ps = psA.tile([128, 512], F32, tag="psA", name="psA")
        sq_ps = psB.tile([128, 512], F32, tag="psB", name="psB")
        for i in range(8):
            nc.tensor.matmul(sum_ps[:1, :TLOC], ones_col[:], src_tiles[i][:],
                             start=(i == 0), stop=(i == 7))
        for i in range(8):
            sq = tmp.tile([128, TLOC], F32, tag="lnsq", name="lnsq")
            nc.scalar.activation(sq[:], src_tiles[i][:], ACTF.Square)
            nc.tensor.matmul(sq_ps[:1, :TLOC], ones_col[:], sq[:],
                             start=(i == 0), stop=(i == 7))
        row = tmps.tile([1, 4 * TLOC], F32, tag="lnrow", name="lnrow", bufs=1)
        mu = row[:, 0:TLOC]
        var = row[:, TLOC:2 * TLOC]
        rstd = row[:, 2 * TLOC:3 * TLOC]
        nrm = row[:, 3 * TLOC:4 * TLOC]
        nc.scalar.mul(mu, sum_ps[:1, :TLOC], 1.0 / E)
        nc.vector.tensor_tensor(nrm, mu, mu, ALU.mult)  # nrm as musq scratch
        nc.vector.scalar_tensor_tensor(var, sq_ps[:1, :TLOC], 1.0 / E, nrm,
                                       ALU.mult, ALU.subtract)
        nc.vector.tensor_scalar(var, var, c_eps[:1, :], None, ALU.add)
        nc.scalar.activation(var, var, ACTF.Ln)
        nc.scalar.activation(rstd, var, ACTF.Exp, scale=-0.5)
        nc.vector.tensor_tensor(nrm, mu, rstd, ALU.mult)
        nc.scalar.mul(nrm, nrm, -1.0)
        outs = []
        for i in range(8):
            a_ps = psA.tile([128, 512], F32, tag="psA", name="psA")
            b_ps = psB.tile([128, 512], F32, tag="psB", name="psB")
            nc.tensor.matmul(a_ps[:, :TLOC], w_rows[i][:], rstd, start=True, stop=True)
            nc.tensor.matmul(b_ps[:, :TLOC], w_rows[i][:], nrm, start=True, stop=False)
            nc.tensor.matmul(b_ps[:, :TLOC], b_rows[i][:], ones_256[:], start=False, stop=True)
            o = acts.tile([128, TLOC], out_dt, tag=f"{out_tag}{i}", name=f"{out_tag}{i}")
            t1 = tmp.tile([128, TLOC], F32, tag="lnt1", name="lnt1")
            nc.vector.tensor_tensor(t1[:], src_tiles[i][:], a_ps[:, :TLOC], ALU.mult)
            nc.vector.tensor_tensor(o[:], t1[:], b_ps[:, :TLOC], ALU.add)
            outs.append(o)
        return outs

    hT = layernorm(xrT, ln1w_r, ln1b_r, "lnout", out_dt=BF16)
    if upto == "ln1":
        finish_featmajor(hT)
        return

    # ---------------- qkv (feature-major, head-pair-packed out) ----------------
    # qkvf[c] layout: [126 part = (parity, d), 4 pairs x 256 tokens]
    qkvf = [acts.tile([P2, FDA], BF16, tag=f"qkvf{c}", name=f"qkvf{c}")
            for c in range(3)]
    qkvw_src = io["qkvw_pk"].rearrange("(n p) f -> n p f", p=128)
    wts = []
    for k in range(8):
        wt = wq.tile([128, 3 * HD], BF16, tag=f"qkvw{k}", name=f"qkvw{k}", bufs=1)
        nc.sync.dma_start(wt[:], qkvw_src[k])
        wts.append(wt)
    for c in range(3):
        for j in range(NPAIR):
            col0 = c * HD + j * P2
            ps = psA.tile([128, 512], F32, tag="psA", name="psA")
            for k in range(8):
                nc.tensor.matmul(ps[:P2, :TLOC], wts[k][:, col0:col0 + P2],
                                 hT[k][:], start=(k == 0), stop=False)
            nc.tensor.matmul(ps[:P2, :TLOC], qkvb_row[:, col0:col0 + P2],
                             ones_256[:], start=False, stop=True)
            if (c * NPAIR + j) % 2 == 0:
                nc.scalar.copy(qkvf[c][:, j * TLOC:(j + 1) * TLOC], ps[:P2, :TLOC])
            else:
                nc.vector.tensor_copy(qkvf[c][:, j * TLOC:(j + 1) * TLOC],
                                      ps[:P2, :TLOC])
    qf, kf, vf = qkvf

    # ---------------- attention (polynomial exp, PE contractions) ----------------
    # phi[n] = q^n (bf16), n = 0..NPOLY
    phi = [ones_phi, qf]
    for n in range(2, NPOLY + 1):
        p = acts.tile([P2, FDA], BF16, tag=f"phi{n}", name=f"phi{n}")
        nc.vector.tensor_tensor(p[:], phi[n - 1][:], qf[:], ALU.mult)
        phi.append(p)

    def eb_matmul(weights, n, rhs_tile):
        gp = psG.tile([128, FDA], F32, tag="psG", name="psG")
        lhs = weights[:, n * P2:(n + 1) * P2]
        for hh in range(FDA // 512):
            nc.tensor.matmul(gp[:P2, hh * 512:(hh + 1) * 512], lhs,
                             rhs_tile[:, hh * 512:(hh + 1) * 512],
                             start=True, stop=True)
        return gp

    # g = sum_n k^n * ((EB/n!)^T @ q^n), Horner descending in n
    acc_g = acts.tile([P2, FDA], BF16, tag="acc_g", name="acc_g")
    for n in range(NPOLY, -1, -1):
        gp = eb_matmul(ebg, n, phi[n])
        if n == NPOLY:
            nc.scalar.copy(acc_g[:], gp[:P2, :])
        else:
            gs = ghp.tile([P2, FDA], BF16, tag="gsb", name="gsb")
            nc.scalar.copy(gs[:], gp[:P2, :])
            nc.vector.tensor_tensor(acc_g[:], acc_g[:], kf[:], ALU.mult)
            nc.vector.tensor_tensor(acc_g[:], acc_g[:], gs[:], ALU.add)

    # u = v / g
    u = acts.tile([P2, FDA], BF16, tag="u_t", name="u_t")
    with nc.allow_low_precision("bf16 attention denominator"):
        nc.vector.reciprocal(u[:], acc_g[:])
    nc.vector.tensor_tensor(u[:], u[:], vf[:], ALU.mult)

    # out = sum_n q^n * ((EB/n!) @ (k^n * u)), ascending accumulation
    out_acc = acts.tile([P2, FDA], BF16, tag="out_acc", name="out_acc")
    zt = acts.tile([P2, FDA], BF16, tag="zt", name="zt")
    for n in range(0, NPOLY + 1):
        if n == 0:
            hp = eb_matmul(ebh, 0, u)
            nc.scalar.copy(out_acc[:], hp[:P2, :])
        else:
            src = u if n == 1 else zt
            nc.vector.tensor_tensor(zt[:], src[:], kf[:], ALU.mult)
            hp = eb_matmul(ebh, n, zt)
            hs = ghp.tile([P2, FDA], BF16, tag="gsb", name="hsb")
            nc.scalar.copy(hs[:], hp[:P2, :])
            tt = tmps.tile([P2, FDA], BF16, tag="phh", name="phh")
            nc.vector.tensor_tensor(tt[:], phi[n][:], hs[:], ALU.mult)
            nc.vector.tensor_tensor(out_acc[:], out_acc[:], tt[:], ALU.add)

    # ---------------- proj + residual ----------------
    pw = []
    pw_src = io["pw_pk"].rearrange("(j p) f -> j p f", p=P2)
    for j in range(NPAIR):
        wt = wq.tile([P2, E], BF16, tag=f"pw{j}", name=f"pw{j}", bufs=1)
        nc.sync.dma_start(wt[:], pw_src[j])
        pw.append(wt)
    xaT = []
    for i in range(8):
        ps = psA.tile([128, 512], F32, tag="psA", name="psA")
        for j in range(NPAIR):
            nc.tensor.matmul(ps[:, :TLOC], pw[j][:, i * 128:(i + 1) * 128],
                             out_acc[:, j * TLOC:(j + 1) * TLOC],
                             start=(j == 0), stop=(j == 3))
        o = acts.tile([128, TLOC], F32, tag=f"xaT{i}", name=f"xaT{i}")
        nc.vector.scalar_tensor_tensor(o[:], ps[:, :TLOC], projb_t[i][:], xrT[i][:],
                                       ALU.add, ALU.add)
        xaT.append(o)

    if upto == "proj":
        finish_featmajor(xaT)
        return

    # ---------------- LN2 ----------------
    h2T = layernorm(xaT, ln2w_r, ln2b_r, "lnout", out_dt=BF16)  # reuses lnout slots

    # ---------------- fc + gelu -> m1g (bf16), then cproj ----------------
    m1g = [m1p.tile([128, TLOC], BF16, tag=f"m1g{j}", name=f"m1g{j}") for j in range(32)]
    fw_g = io["fw_t"].rearrange("(k p) (g f) -> p k g f", p=128, f=256)  # [128,8,16,256]
    for jg in range(16):          # groups of 2 j-tiles
        fwg = wf.tile([128, 8, 256], BF16, tag="fwg", name="fwg", bufs=4)
        nc.sync.dma_start(fwg[:], fw_g[:, :, jg, :])
        for jj in range(2):
            j = jg * 2 + jj
            fps = psB.tile([128, 512], F32, tag="psB", name="psB")
            for k in range(8):
                nc.tensor.matmul(fps[:, :TLOC], fwg[:, k, jj * 128:(jj + 1) * 128],
                                 h2T[k][:], start=(k == 0), stop=(k == 7))
            gelu_f = ACTF.Tanh if knobs.get("sim_tanh") else ACTF.Gelu
            nc.scalar.activation(m1g[j][:], fps[:, :TLOC], gelu_f, bias=fcb_t[j][:])
    # cproj: e-outer, contract over 32 j-tiles
    cw_src = io["cw_te"].rearrange("(e p) f -> e p f", p=128)    # [8,128,4096]
    for e in range(8):
        cps = psG.tile([128, FDA], F32, tag="psG", name="cpp")
        for half in range(2):
            cwt = wc.tile([128, E4 // 2], BF16, tag="cwt", name="cwt")
            nc.sync.dma_start(cwt[:], cw_src[e, :, half * 2048:(half + 1) * 2048])
            for jj in range(16):
                j = half * 16 + jj
                nc.tensor.matmul(cps[:, :TLOC], cwt[:, jj * 128:(jj + 1) * 128],
                                 m1g[j][:], start=(j == 0), stop=(j == 31))
        yT = tmp.tile([128, TLOC], F32, tag="yT", name="yT")
        nc.vector.scalar_tensor_tensor(yT[:], cps[:, :TLOC], cprojb_t[e][:], xaT[e][:],
                                       ALU.add, ALU.add)
        for m in range(NT):
            ps = psB.tile([128, 512], F32, tag="psB", name="psB")
            nc.tensor.transpose(ps[:, :128], yT[:, m * 128:(m + 1) * 128], ident[:])
            ysb = tmp.tile([128, 128], F32, tag="ysb", name="ysb")
            nc.scalar.copy(ysb[:], ps[:, :128])
            nc.sync.dma_start(
                io["y"].rearrange("(n p) f -> n p f", p=128)[m, :, e * 128:(e + 1) * 128],
                ysb[:])


def build(knobs=None):
    from contextlib import ExitStack
    knobs = knobs or {}
    nc = bacc.Bacc("TRN2", target_bir_lowering=False, debug=False)
    io = {}

    def din(name, shape, dt=F32):
        io[name] = nc.dram_tensor(name, shape, dt, kind="ExternalInput").ap()

    din("x", [TLOC, 512])
    din("qkvw_pk", [E, 3 * HD], BF16)
    din("qkvb_pk", [3 * HD])
    din("ebg", [P2, (NPOLY + 1) * P2], BF16)
    din("ebh", [P2, (NPOLY + 1) * P2], BF16)
    din("pw_pk", [HD, E], BF16)
    din("fw_t", [E, E4], BF16)
    din("cw_te", [E, E4], BF16)     # per-e k-major packing, see host_prep
    din("invfreq", [512])
    din("ln1w", [E]); din("ln1b", [E])
    din("ln2w", [E]); din("ln2b", [E])
    din("projb", [E]); din("fcb", [E4]); din("cprojb", [E])
    io["y"] = nc.dram_tensor("y", [TLOC, E], F32, kind="ExternalOutput").ap()

    with tile.TileContext(nc) as tc:
        with ExitStack() as ctx:
            emit(nc, tc, io, ctx, knobs)
    nc.compile()
    return nc


def host_prep(inputs):
    x = np.asarray(inputs["x"], np.float32).reshape(B * T, E // 2)
    qkv_w = np.asarray(inputs["qkv_w"], np.float32)
    qkv_b = np.asarray(inputs["qkv_b"], np.float32)
    rel_pos = np.asarray(inputs["rel_pos"], np.float32)
    proj_w = np.asarray(inputs["proj_w"], np.float32)
    fc_w = np.asarray(inputs["fc_w"], np.float32)
    cproj_w = np.asarray(inputs["cproj_w"], np.float32)

    inv_freq = (1.0 / 10000.0 ** (np.arange(0, E, 2, dtype=np.float32) / E)).astype(np.float32)

    # head-pair packing permutation: new (c, j, parity, d) <- old (c, h=2j+parity, d)
    colperm = np.empty(3 * HD, np.int64)
    for c in range(3):
        for j in range(NPAIR):
            for par in range(2):
                h = 2 * j + par
                dst = c * HD + j * P2 + par * D
                src = c * HD + h * D
                colperm[dst:dst + D] = np.arange(src, src + D)
    qkvw_pk = np.ascontiguousarray(qkv_w.T[:, colperm].astype(ml_dtypes.bfloat16))
    qkvb_pk = np.ascontiguousarray(qkv_b[colperm])

    perm = np.arange(-W, W + 1) % D
    EB = np.exp(rel_pos[perm]).astype(np.float64)        # [d, v]
    EBbd = np.zeros((P2, P2))
    EBbd[:D, :D] = EB
    EBbd[D:, D:] = EB
    ebg = np.concatenate(
        [EBbd / math.factorial(n) for n in range(NPOLY + 1)], axis=1)
    ebh = np.concatenate(
        [EBbd.T / math.factorial(n) for n in range(NPOLY + 1)], axis=1)

    rowperm = colperm[:HD]    # same (j, parity, d) <- (h, d) reorder
    pw_pk = np.ascontiguousarray(proj_w.T[rowperm].astype(ml_dtypes.bfloat16))

    # cw_te[e]: [4096, 128] column-block e of cproj_w.T, repacked so SBUF tile
    # [128, 4096] holds k-tile j at cols j*128:(j+1)*128
    cw_t = cproj_w.T.astype(ml_dtypes.bfloat16)          # [4096, 1024]
    cw_te = np.empty((E, E4), ml_dtypes.bfloat16)
    for e in range(8):
        blk = cw_t[:, e * 128:(e + 1) * 128]             # [4096, 128]
        cw_te[e * 128:(e + 1) * 128] = (
            blk.reshape(32, 128, 128).transpose(1, 0, 2).reshape(128, E4))

    common = {
        "qkvw_pk": qkvw_pk,
        "qkvb_pk": qkvb_pk,
        "ebg": ebg.astype(ml_dtypes.bfloat16),
        "ebh": ebh.astype(ml_dtypes.bfloat16),
        "pw_pk": pw_pk,
        "fw_t": np.ascontiguousarray(fc_w.T.astype(ml_dtypes.bfloat16)),
        "cw_te": cw_te,
        "invfreq": inv_freq,
        "ln1w": np.asarray(inputs["ln1_w"], np.float32),
        "ln1b": np.asarray(inputs["ln1_b"], np.float32),
        "ln2w": np.asarray(inputs["ln2_w"], np.float32),
        "ln2b": np.asarray(inputs["ln2_b"], np.float32),
        "projb": np.asarray(inputs["proj_b"], np.float32),
        "fcb": np.asarray(inputs["fc_b"], np.float32),
        "cprojb": np.asarray(inputs["cproj_b"], np.float32),
    }
    in_maps = []
    for c in range(NCORES):
        m = dict(common)
        m["x"] = np.ascontiguousarray(x[c * TLOC:(c + 1) * TLOC])
        in_maps.append(m)
    return in_maps


def kernel(**inputs):
    nc = build()
    in_maps = host_prep(inputs)
    res = run_bass_kernel_spmd(nc, in_maps, list(range(NCORES))).results
    y = np.concatenate([res[c]["y"] for c in range(NCORES)], axis=0)
    return y.reshape(B, T, E)


# revision 17
# speedup vs baseline: 3.9625x; 1.2905x over previous
"""Trainium2 Bass kernel for nn_Block_70093866270826.

Sharding: token-data-parallel across 8 cores (the entire block is per-token
math: rotary, LN, per-token windowed attention, MLP — no cross-token mixing),
so each core processes 256 of the 2048 tokens with full weights. No
collectives.

Attention: the per-token softmax over exp(q_d*k_v + B_dv) is evaluated via a
truncated-exp rank decomposition: exp(q*k) = sum_n (q^n/n!) k^n, so both the
softmax denominator g[t,v] = sum_d exp(.)e^B and the value application
out[t,d] = sum_v exp(.)e^B u[t,v] become PE matmuls against the constant
(e^B / n!) matrices, with Horner/ascending accumulation over n on the DVE in
bf16 (2x mode). Heads are packed two per 126-partition tile with
block-diagonal weight matrices. Truncation error at N=8 is ~3e-4 relative,
far below the bf16 noise floor.

Layouts: feature-major [feat_part, tok_free] for the matmul chain; the
attention island is feature-major too ([126 = 2*63 head-pair rows,
4 pairs x 256 tokens] tiles), so no transposes are needed between qkv,
attention, and proj.
"""
import math
import sys

sys.path.insert(0, "/opt/trn_rl_repo")

import ml_dtypes
import numpy as np

import concourse.bass as bass
import concourse.tile as tile
from concourse import bacc, mybir
from concourse.bass import AP
from concourse.bass_utils import run_bass_kernel_spmd
from concourse.masks import make_identity

F32 = mybir.dt.float32
F32R = mybir.dt.float32r
BF16 = mybir.dt.bfloat16
ALU = mybir.AluOpType
ACTF = mybir.ActivationFunctionType
AXX = mybir.AxisListType.X

B, T, E, H, W = 2, 1024, 1024, 8, 31
D = 2 * W + 1            # 63
P2 = 2 * D               # 126 partitions = head pair
NPAIR = H // 2           # 4
HD = H * D               # 504
E4 = 4 * E
NCORES = 8
TLOC = (B * T) // NCORES  # 256
NT = TLOC // 128          # 2
FDA = NPAIR * TLOC        # 1024: attention tile free size
NPOLY = 6                 # exp() Taylor degree (rel err ~3e-3, bf16-dominated)
PI = float(np.pi)
TWO_PI = float(2 * np.pi)
EPS = 1e-5


def emit(nc, tc, io, ctx, knobs):
    iters = knobs.get("iters", 0)
    upto = knobs.get("upto", "full")
    if iters:
        ctx.enter_context(tc.For_i(0, iters, 1))
    consts = ctx.enter_context(tc.tile_pool(name="consts", bufs=1))
    acts = ctx.enter_context(tc.tile_pool(name="acts", bufs=1))
    wq = ctx.enter_context(tc.tile_pool(name="wq", bufs=3))
    wf = ctx.enter_context(tc.tile_pool(name="wf", bufs=1))
    wc = ctx.enter_context(tc.tile_pool(name="wc", bufs=4))
    m1p = ctx.enter_context(tc.tile_pool(name="m1p", bufs=1))
    tmp = ctx.enter_context(tc.tile_pool(name="tmp", bufs=2))
    tmps = ctx.enter_context(tc.tile_pool(name="tmps", bufs=3))
    ghp = ctx.enter_context(tc.tile_pool(name="ghp", bufs=3))
    # PSUM: psA/psB one bank x2 bufs, psG two banks x2 bufs = 8 banks exactly.
    psA = ctx.enter_context(tc.tile_pool(name="psA", bufs=2, space="PSUM"))
    psB = ctx.enter_context(tc.tile_pool(name="psB", bufs=2, space="PSUM"))
    psG = ctx.enter_context(tc.tile_pool(name="psG", bufs=2, space="PSUM"))

    # ---------------- input x first (everything waits on it) ----------------
    xtiles = []
    for m in range(NT):
        xtile = tmp.tile([128, 512], F32, tag=f"xin{m}", name=f"xin{m}", bufs=1)
        for hh in range(2):
            nc.sync.dma_start(
                xtile[:, hh * 256:(hh + 1) * 256],
                io["x"].rearrange("(n p) f -> n p f", p=128)[m, :, hh * 256:(hh + 1) * 256])
        xtiles.append(xtile)

    # ---------------- constants (batched DMAs) ----------------
    ident = consts.tile([128, 128], F32, name='ident')
    make_identity(nc, ident[:])

    # per-partition vectors, one [128, 52] tile: invfreq(4) projb(8) fcb(32) cprojb(8)
    cvec = consts.tile([128, 52], F32, name='cvec')
    nc.sync.dma_start(cvec[:], io["cvec"])
    invfreq_t = [cvec[:, i:i + 1] for i in range(0, 4)]
    projb_t = [cvec[:, 4 + i:5 + i] for i in range(8)]
    fcb_t = [cvec[:, 12 + i:13 + i] for i in range(32)]
    cprojb_t = [cvec[:, 44 + i:45 + i] for i in range(8)]

    # row vectors, one [1, 4E] tile: ln1w ln1b ln2w ln2b
    crow = consts.tile([1, 4 * E], F32R, name='crow')
    nc.sync.dma_start(crow[:], io["crow"].rearrange("(o f) -> o f", o=1))
    ln1w_r = [crow[:, 0 * E + i * 128:0 * E + (i + 1) * 128] for i in range(8)]
    ln1b_r = [crow[:, 1 * E + i * 128:1 * E + (i + 1) * 128] for i in range(8)]
    ln2w_r = [crow[:, 2 * E + i * 128:2 * E + (i + 1) * 128] for i in range(8)]
    ln2b_r = [crow[:, 3 * E + i * 128:3 * E + (i + 1) * 128] for i in range(8)]

    ebgh = consts.tile([P2, 2 * (NPOLY + 1) * P2], BF16, name='ebgh')
    nc.sync.dma_start(ebgh[:], io["ebgh"])
    NEB = (NPOLY + 1) * P2
    ebg = ebgh[:, :NEB]
    ebh = ebgh[:, NEB:]

    qkvb_row = consts.tile([1, 3 * HD], F32R, name='qkvb_row')
    nc.sync.dma_start(qkvb_row[:], io["qkvb_pk"].rearrange("(o f) -> o f", o=1))

    def sconst(val, name):
        t = consts.tile([128, 1], F32, tag=name)
        nc.vector.memset(t[:], float(val))
        return t

    c_pi = sconst(PI, "c_pi")
    c_negpi = sconst(-PI, "c_negpi")
    c_halfpi = sconst(PI / 2, "c_halfpi")
    c_neg3halfpi = sconst(-1.5 * PI, "c_neg3halfpi")
    c_n2pi = sconst(-TWO_PI, "c_n2pi")
    c_p2pi = sconst(TWO_PI, "c_p2pi")
    c_eps = sconst(EPS, "c_eps")
    ones_colf = sconst(1.0, "ones_colf")
    ones_col = consts.tile([128, 1], F32R, tag="ones_col", name="ones_col")
    nc.scalar.copy(ones_col[:], ones_colf[:])
    ones_256f = consts.tile([1, TLOC], F32, tag="ones_256f", name="ones_256f")
    nc.vector.memset(ones_256f[:], 1.0)
    ones_256 = consts.tile([1, TLOC], F32R, tag="ones_256", name="ones_256")
    nc.scalar.copy(ones_256[:], ones_256f[:])
    ones_phi = consts.tile([P2, FDA], BF16, tag="ones_phi", name="ones_phi")
    nc.vector.memset(ones_phi[:], 1.0)

    if upto == "noop":
        for m in range(NT):
            z = tmp.tile([128, E], F32, tag="znoop", name="znoop")
            nc.vector.memset(z[:], 0.0)
            nc.sync.dma_start(io["y"].rearrange("(n p) f -> n p f", p=128)[m], z[:])
        return

    # ---------------- transpose x ----------------
    xT = [acts.tile([128, TLOC], F32, tag=f"xaT{i}", name=f"xT{i}") for i in range(4)]
    for m in range(NT):
        xtile = xtiles[m]
        for i in range(4):
            ps = psA.tile([128, 512], F32, tag="psA", name="psA")
            nc.tensor.transpose(ps[:, :128], xtile[:, i * 128:(i + 1) * 128], ident[:])
            nc.scalar.copy(xT[i][:, m * 128:(m + 1) * 128], ps[:, :128])

    # ---------------- rotary ----------------
    xrT = [acts.tile([128, TLOC], F32R, tag=f"xrT{i}", name=f"xrT{i}") for i in range(8)]
    for i in range(4):
        ang = tmp.tile([128, TLOC], F32, tag="ang", name="ang")
        nc.vector.tensor_scalar(ang[:], xT[i][:], invfreq_t[i][:], None, ALU.mult)
        m1 = tmp.tile([128, TLOC], F32, tag="m1", name="m1")
        m2 = tmp.tile([128, TLOC], F32, tag="m2", name="m2")
        r = tmp.tile([128, TLOC], F32, tag="r", name="r")
        nc.vector.tensor_scalar(m1[:], ang[:], c_pi[:], None, ALU.is_gt)
        nc.vector.tensor_scalar(m2[:], ang[:], c_negpi[:], None, ALU.is_lt)
        nc.vector.scalar_tensor_tensor(r[:], m1[:], c_n2pi[:], ang[:], ALU.mult, ALU.add)
        nc.vector.scalar_tensor_tensor(r[:], m2[:], c_p2pi[:], r[:], ALU.mult, ALU.add)
        nc.scalar.activation(xrT[i][:], r[:], ACTF.Sin)
        nc.vector.tensor_scalar(m1[:], ang[:], c_halfpi[:], None, ALU.is_gt)
        nc.vector.tensor_scalar(m2[:], ang[:], c_neg3halfpi[:], None, ALU.is_lt)
        nc.vector.scalar_tensor_tensor(r[:], m1[:], c_n2pi[:], ang[:], ALU.mult, ALU.add)
        nc.vector.scalar_tensor_tensor(r[:], m2[:], c_p2pi[:], r[:], ALU.mult, ALU.add)
        nc.scalar.activation(xrT[4 + i][:], r[:], ACTF.Sin, bias=c_halfpi[:])

    def finish_featmajor(tiles8):
        for e in range(8):
            src_t = tiles8[e]
            sap = src_t[:].bitcast(F32) if src_t.dtype in (F32R,) else (
                src_t[:] if src_t.dtype == F32 else src_t[:].bitcast(F32))
            for m in range(NT):
                ps = psA.tile([128, 512], F32, tag="psA", name="psAf")
                nc.tensor.transpose(ps[:, :128], sap[:, m * 128:(m + 1) * 128], ident[:])
                ysb = tmp.tile([128, 128], F32, tag="ysb", name="ysbf")
                nc.scalar.copy(ysb[:], ps[:, :128])
                nc.sync.dma_start(
                    io["y"].rearrange("(n p) f -> n p f", p=128)[m, :, e * 128:(e + 1) * 128],
                    ysb[:])

    if upto == "rotary":
        finish_featmajor(xrT)
        return

    # ---------------- layernorm helper (feat-major over 8 tiles) ----------------
    def layernorm(src_tiles, w_rows, b_rows, out_tag, out_dt=BF16):
        sum_ps = psA.tile([128, 512], F32, tag="psA", name="psA")
        sq_ps = psB.tile([128, 512], F32, tag="psB", name="psB")
        for i in range(8):
            nc.tensor.matmul(sum_ps[:1, :TLOC], ones_col[:], src_tiles[i][:],
                             start=(i == 0), stop=(i == 7))
        for i in range(8):
            sq = tmp.tile([128, TLOC], F32R, tag="lnsq", name="lnsq")
            nc.scalar.activation(sq[:], src_tiles[i][:].bitcast(F32), ACTF.Square)
            nc.tensor.matmul(sq_ps[:1, :TLOC], ones_col[:], sq[:],
                             start=(i == 0), stop=(i == 7))
        row = tmps.tile([1, 4 * TLOC], F32R, tag="lnrow", name="lnrow", bufs=1)
        mu = row[:, 0:TLOC]
        var = row[:, TLOC:2 * TLOC]
        rstd = row[:, 2 * TLOC:3 * TLOC]
        nrm = row[:, 3 * TLOC:4 * TLOC]
        _f = lambda ap: ap.bitcast(F32)
        nc.scalar.mul(mu, sum_ps[:1, :TLOC], 1.0 / E)
        nc.vector.tensor_tensor(nrm, _f(mu), _f(mu), ALU.mult)  # nrm as musq scratch
        nc.vector.scalar_tensor_tensor(var, sq_ps[:1, :TLOC], 1.0 / E, _f(nrm),
                                       ALU.mult, ALU.subtract)
        nc.vector.tensor_scalar(var, _f(var), c_eps[:1, :], None, ALU.add)
        nc.scalar.activation(var, _f(var), ACTF.Ln)
        nc.scalar.activation(rstd, _f(var), ACTF.Exp, scale=-0.5)
        nc.vector.tensor_tensor(nrm, _f(mu), _f(rstd), ALU.mult)
        nc.scalar.mul(nrm, _f(nrm), -1.0)
        outs = []
        for i in range(8):
            a_ps = psA.tile([128, 512], F32, tag="psA", name="psA")
            b_ps = psB.tile([128, 512], F32, tag="psB", name="psB")
            nc.tensor.matmul(a_ps[:, :TLOC], w_rows[i][:], rstd,
                             start=True, stop=True)
            nc.tensor.matmul(b_ps[:, :TLOC], w_rows[i][:], nrm,
                             start=True, stop=False)
            nc.tensor.matmul(b_ps[:, :TLOC], b_rows[i][:], ones_256[:],
                             start=False, stop=True)
            o = acts.tile([128, TLOC], out_dt, tag=f"{out_tag}{i}", name=f"{out_tag}{i}")
            t1 = tmp.tile([128, TLOC], F32, tag="lnt1", name="lnt1")
            nc.vector.tensor_tensor(t1[:], src_tiles[i][:].bitcast(F32),
                                    a_ps[:, :TLOC], ALU.mult)
            nc.vector.tensor_tensor(o[:], t1[:], b_ps[:, :TLOC], ALU.add)
            outs.append(o)
        return outs

    hT = layernorm(xrT, ln1w_r, ln1b_r, "lnout", out_dt=BF16)
    if upto == "ln1":
        finish_featmajor(hT)
        return

    # ---------------- qkv (feature-major, head-pair-packed out) ----------------
    # qkvf[c] layout: [126 part = (parity, d), 4 pairs x 256 tokens]
    qkvf = [acts.tile([P2, FDA], BF16, tag=f"qkvf{c}", name=f"qkvf{c}")
            for c in range(3)]
    qkvw_src = io["qkvw_pk"].rearrange("(n p) f -> n p f", p=128)
    wts = []
    for k in range(8):
        wt = wq.tile([128, 3 * HD], BF16, tag=f"qkvw{k}", name=f"qkvw{k}", bufs=1)
        nc.sync.dma_start(wt[:], qkvw_src[k])
        wts.append(wt)
    for c in range(3):
        for j in range(NPAIR):
            col0 = c * HD + j * P2
            ps = psA.tile([128, 512], F32, tag="psA", name="psA")
            for k in range(8):
                nc.tensor.matmul(ps[:P2, :TLOC], wts[k][:, col0:col0 + P2],
                                 hT[k][:], start=(k == 0), stop=False)
            nc.tensor.matmul(ps[:P2, :TLOC], qkvb_row[:, col0:col0 + P2],
                             ones_256[:], start=False, stop=True)
            if (c * NPAIR + j) % 2 == 0:
                nc.scalar.copy(qkvf[c][:, j * TLOC:(j + 1) * TLOC], ps[:P2, :TLOC])
            else:
                nc.vector.tensor_copy(qkvf[c][:, j * TLOC:(j + 1) * TLOC],
                                      ps[:P2, :TLOC])
    qf, kf, vf = qkvf

    # ---------------- attention (polynomial exp, PE contractions) ----------------
    # phi[n] = q^n (bf16), n = 0..NPOLY
    phi = [ones_phi, qf]
    for n in range(2, NPOLY + 1):
        p = acts.tile([P2, FDA], BF16, tag=f"phi{n}", name=f"phi{n}")
        nc.vector.tensor_tensor(p[:], phi[n - 1][:], qf[:], ALU.mult)
        phi.append(p)

    def eb_matmul(weights, n, rhs_tile):
        gp = psG.tile([128, FDA], F32, tag="psG", name="psG")
        lhs = weights[:, n * P2:(n + 1) * P2]
        for hh in range(FDA // 512):
            nc.tensor.matmul(gp[:P2, hh * 512:(hh + 1) * 512], lhs,
                             rhs_tile[:, hh * 512:(hh + 1) * 512],
                             start=True, stop=True)
        return gp

    # g = sum_n k^n * ((EB/n!)^T @ q^n), Horner descending in n
    acc_g = acts.tile([P2, FDA], BF16, tag="acc_g", name="acc_g")
    for n in range(NPOLY, -1, -1):
        gp = eb_matmul(ebg, n, phi[n])
        if n == NPOLY:
            nc.scalar.copy(acc_g[:], gp[:P2, :])
        else:
            gs = ghp.tile([P2, FDA], BF16, tag="gsb", name="gsb")
            nc.scalar.copy(gs[:], gp[:P2, :])
            nc.vector.tensor_tensor(acc_g[:], acc_g[:], kf[:], ALU.mult)
            nc.vector.tensor_tensor(acc_g[:], acc_g[:], gs[:], ALU.add)

    # u = v / g
    u = acts.tile([P2, FDA], BF16, tag="u_t", name="u_t")
    with nc.allow_low_precision("bf16 attention denominator"):
        nc.vector.reciprocal(u[:], acc_g[:])
    nc.vector.tensor_tensor(u[:], u[:], vf[:], ALU.mult)

    # out = sum_n q^n * ((EB/n!) @ (k^n * u)), ascending accumulation.
    # The zt chain (k^n * u) runs 2 levels ahead of the phh/add consumers so
    # the DVE never stalls on the PE->ACT copy round-trip of H_n.
    out_acc = acts.tile([P2, FDA], BF16, tag="out_acc", name="out_acc")
    zts = [u]
    hss = [None] * (NPOLY + 1)
    LAG = 2

    def emit_produce(n):
        if n > NPOLY:
            return
        if n >= 1:
            zt = ghp.tile([P2, FDA], BF16, tag="zt", name=f"zt{n}", bufs=LAG + 2)
            nc.vector.tensor_tensor(zt[:], zts[n - 1][:], kf[:], ALU.mult)
            zts.append(zt)
        hp = eb_matmul(ebh, n, zts[n])
        if n == 0:
            nc.scalar.copy(out_acc[:], hp[:P2, :])
        else:
            hs = ghp.tile([P2, FDA], BF16, tag="gsb", name="hsb")
            nc.scalar.copy(hs[:], hp[:P2, :])
            hss[n] = hs

    def emit_consume(n):
        if not (1 <= n <= NPOLY):
            return
        tt = tmps.tile([P2, FDA], BF16, tag="phh", name="phh")
        nc.vector.tensor_tensor(tt[:], phi[n][:], hss[n][:], ALU.mult)
        nc.vector.tensor_tensor(out_acc[:], out_acc[:], tt[:], ALU.add)

    for n in range(0, NPOLY + 1 + LAG):
        emit_produce(n)
        emit_consume(n - LAG)

    # ---------------- proj + residual ----------------
    pw = []
    pw_src = io["pw_pk"].rearrange("(j p) f -> j p f", p=P2)
    for j in range(NPAIR):
        wt = wq.tile([P2, E], BF16, tag=f"pw{j}", name=f"pw{j}", bufs=1)
        nc.sync.dma_start(wt[:], pw_src[j])
        pw.append(wt)
    xaT = []
    for i in range(8):
        ps = psA.tile([128, 512], F32, tag="psA", name="psA")
        for j in range(NPAIR):
            nc.tensor.matmul(ps[:, :TLOC], pw[j][:, i * 128:(i + 1) * 128],
                             out_acc[:, j * TLOC:(j + 1) * TLOC],
                             start=(j == 0), stop=(j == 3))
        o = acts.tile([128, TLOC], F32R, tag=f"xaT{i}", name=f"xaT{i}")
        nc.vector.scalar_tensor_tensor(o[:], ps[:, :TLOC], projb_t[i][:],
                                       xrT[i][:].bitcast(F32), ALU.add, ALU.add)
        xaT.append(o)

    if upto == "proj":
        finish_featmajor(xaT)
        return

    # ---------------- LN2 ----------------
    h2T = layernorm(xaT, ln2w_r, ln2b_r, "lnout", out_dt=BF16)  # reuses lnout slots

    # ---------------- fc + gelu -> m1g (bf16), then cproj ----------------
    m1g = [m1p.tile([128, TLOC], BF16, tag=f"m1g{j}", name=f"m1g{j}") for j in range(32)]
    fw_g = io["fw_t"].rearrange("(k p) (g f) -> p k g f", p=128, f=256)  # [128,8,16,256]
    for jg in range(16):          # groups of 2 j-tiles
        fwg = wf.tile([128, 8, 256], BF16, tag="fwg", name="fwg", bufs=4)
        nc.sync.dma_start(fwg[:], fw_g[:, :, jg, :])
        for jj in range(2):
            j = jg * 2 + jj
            fps = psB.tile([128, 512], F32, tag="psB", name="psB")
            for k in range(8):
                nc.tensor.matmul(fps[:, :TLOC], fwg[:, k, jj * 128:(jj + 1) * 128],
                                 h2T[k][:], start=(k == 0), stop=(k == 7))
            gelu_f = ACTF.Tanh if knobs.get("sim_tanh") else ACTF.Gelu
            nc.scalar.activation(m1g[j][:], fps[:, :TLOC], gelu_f, bias=fcb_t[j][:])
    # cproj: e-outer, contract over 32 j-tiles
    cw_src = io["cw_te"].rearrange("(e p) f -> e p f", p=128)    # [8,128,4096]
    for e in range(8):
        cps = psG.tile([128, FDA], F32, tag="psG", name="cpp")
        for half in range(2):
            cwt = wc.tile([128, E4 // 2], BF16, tag="cwt", name="cwt")
            nc.sync.dma_start(cwt[:], cw_src[e, :, half * 2048:(half + 1) * 2048])
            for jj in range(16):
                j = half * 16 + jj
                nc.tensor.matmul(cps[:, :TLOC], cwt[:, jj * 128:(jj + 1) * 128],
                                 m1g[j][:], start=(j == 0), stop=(j == 31))
        yT = tmp.tile([128, TLOC], F32, tag="yT", name="yT")
        nc.vector.scalar_tensor_tensor(yT[:], cps[:, :TLOC], cprojb_t[e][:],
                                       xaT[e][:].bitcast(F32), ALU.add, ALU.add)
        for m in range(NT):
            ps = psB.tile([128, 512], F32, tag="psB", name="psB")
            nc.tensor.transpose(ps[:, :128], yT[:, m * 128:(m + 1) * 128], ident[:])
            ysb = tmp.tile([128, 128], F32, tag="ysb", name="ysb")
            nc.scalar.copy(ysb[:], ps[:, :128])
            nc.sync.dma_start(
                io["y"].rearrange("(n p) f -> n p f", p=128)[m, :, e * 128:(e + 1) * 128],
                ysb[:])


def build(knobs=None):
    from contextlib import ExitStack
    knobs = knobs or {}
    nc = bacc.Bacc("TRN2", target_bir_lowering=False, debug=False)
    io = {}

    def din(name, shape, dt=F32):
        io[name] = nc.dram_tensor(name, shape, dt, kind="ExternalInput").ap()

    din("x", [TLOC, 512])
    din("qkvw_pk", [E, 3 * HD], BF16)
    din("qkvb_pk", [3 * HD], F32R)
    din("ebgh", [P2, 2 * (NPOLY + 1) * P2], BF16)
    din("pw_pk", [HD, E], BF16)
    din("fw_t", [E, E4], BF16)
    din("cw_te", [E, E4], BF16)     # per-e k-major packing, see host_prep
    din("cvec", [128, 52])          # invfreq | projb | fcb | cprojb columns
    din("crow", [4 * E], F32R)      # ln1w | ln1b | ln2w | ln2b
    io["y"] = nc.dram_tensor("y", [TLOC, E], F32, kind="ExternalOutput").ap()

    with tile.TileContext(nc) as tc:
        with ExitStack() as ctx:
            emit(nc, tc, io, ctx, knobs)
    nc.compile()
    return nc


def host_prep(inputs):
    x = np.asarray(inputs["x"], np.float32).reshape(B * T, E // 2)
    qkv_w = np.asarray(inputs["qkv_w"], np.float32)
    qkv_b = np.asarray(inputs["qkv_b"], np.float32)
    rel_pos = np.asarray(inputs["rel_pos"], np.float32)
    proj_w = np.asarray(inputs["proj_w"], np.float32)
    fc_w = np.asarray(inputs["fc_w"], np.float32)
    cproj_w = np.asarray(inputs["cproj_w"], np.float32)

    inv_freq = (1.0 / 10000.0 ** (np.arange(0, E, 2, dtype=np.float32) / E)).astype(np.float32)

    # head-pair packing permutation: new (c, j, parity, d) <- old (c, h=2j+parity, d)
    colperm = np.empty(3 * HD, np.int64)
    for c in range(3):
        for j in range(NPAIR):
            for par in range(2):
                h = 2 * j + par
                dst = c * HD + j * P2 + par * D
                src = c * HD + h * D
                colperm[dst:dst + D] = np.arange(src, src + D)
    qkvw_pk = np.ascontiguousarray(qkv_w.T[:, colperm].astype(ml_dtypes.bfloat16))
    qkvb_pk = np.ascontiguousarray(qkv_b[colperm])

    perm = np.arange(-W, W + 1) % D
    EB = np.exp(rel_pos[perm]).astype(np.float64)        # [d, v]
    EBbd = np.zeros((P2, P2))
    EBbd[:D, :D] = EB
    EBbd[D:, D:] = EB
    ebg = np.concatenate(
        [EBbd / math.factorial(n) for n in range(NPOLY + 1)], axis=1)
    ebh = np.concatenate(
        [EBbd.T / math.factorial(n) for n in range(NPOLY + 1)], axis=1)

    rowperm = colperm[:HD]    # same (j, parity, d) <- (h, d) reorder
    pw_pk = np.ascontiguousarray(proj_w.T[rowperm].astype(ml_dtypes.bfloat16))

    # cw_te[e]: [4096, 128] column-block e of cproj_w.T, repacked so SBUF tile
    # [128, 4096] holds k-tile j at cols j*128:(j+1)*128
    cw_t = cproj_w.T.astype(ml_dtypes.bfloat16)          # [4096, 1024]
    cw_te = np.empty((E, E4), ml_dtypes.bfloat16)
    for e in range(8):
        blk = cw_t[:, e * 128:(e + 1) * 128]             # [4096, 128]
        cw_te[e * 128:(e + 1) * 128] = (
            blk.reshape(32, 128, 128).transpose(1, 0, 2).reshape(128, E4))

    cvec = np.zeros((128, 52), np.float32)
    cvec[:, 0:4] = inv_freq.reshape(4, 128).T
    cvec[:, 4:12] = np.asarray(inputs["proj_b"], np.float32).reshape(8, 128).T
    cvec[:, 12:44] = np.asarray(inputs["fc_b"], np.float32).reshape(32, 128).T
    cvec[:, 44:52] = np.asarray(inputs["cproj_b"], np.float32).reshape(8, 128).T
    crow = np.concatenate([
        np.asarray(inputs["ln1_w"], np.float32),
        np.asarray(inputs["ln1_b"], np.float32),
        np.asarray(inputs["ln2_w"], np.float32),
        np.asarray(inputs["ln2_b"], np.float32)])

    common = {
        "qkvw_pk": qkvw_pk,
        "qkvb_pk": qkvb_pk,
        "ebgh": np.concatenate([ebg, ebh], axis=1).astype(ml_dtypes.bfloat16),
        "pw_pk": pw_pk,
        "fw_t": np.ascontiguousarray(fc_w.T.astype(ml_dtypes.bfloat16)),
        "cw_te": cw_te,
        "cvec": cvec,
        "crow": crow,
    }
    in_maps = []
    for c in range(NCORES):
        m = dict(common)
        m["x"] = np.ascontiguousarray(x[c * TLOC:(c + 1) * TLOC])
        in_maps.append(m)
    return in_maps


def kernel(**inputs):
    nc = build()
    in_maps = host_prep(inputs)
    res = run_bass_kernel_spmd(nc, in_maps, list(range(NCORES))).results
    y = np.concatenate([res[c]["y"] for c in range(NCORES)], axis=0)
    return y.reshape(B, T, E)


# revision 18
# speedup vs baseline: 4.3583x; 1.0999x over previous
"""Trainium2 Bass kernel for nn_Block_70093866270826.

Sharding: token-data-parallel across 8 cores (the entire block is per-token
math: rotary, LN, per-token windowed attention, MLP — no cross-token mixing),
so each core processes 256 of the 2048 tokens with full weights. No
collectives.

Attention: the per-token softmax over exp(q_d*k_v + B_dv) is evaluated via a
truncated-exp rank decomposition: exp(q*k) = sum_n (q^n/n!) k^n, so both the
softmax denominator g[t,v] = sum_d exp(.)e^B and the value application
out[t,d] = sum_v exp(.)e^B u[t,v] become PE matmuls against the constant
(e^B / n!) matrices, with Horner/ascending accumulation over n on the DVE in
bf16 (2x mode). Heads are packed two per 126-partition tile with
block-diagonal weight matrices. Truncation error at N=8 is ~3e-4 relative,
far below the bf16 noise floor.

Layouts: feature-major [feat_part, tok_free] for the matmul chain; the
attention island is feature-major too ([126 = 2*63 head-pair rows,
4 pairs x 256 tokens] tiles), so no transposes are needed between qkv,
attention, and proj.
"""
import math
import sys

sys.path.insert(0, "/opt/trn_rl_repo")

import ml_dtypes
import numpy as np

import concourse.bass as bass
import concourse.tile as tile
from concourse import bacc, mybir
from concourse.bass import AP
from concourse.bass_utils import run_bass_kernel_spmd
from concourse.masks import make_identity

F32 = mybir.dt.float32
F32R = mybir.dt.float32r
BF16 = mybir.dt.bfloat16
ALU = mybir.AluOpType
ACTF = mybir.ActivationFunctionType
AXX = mybir.AxisListType.X

B, T, E, H, W = 2, 1024, 1024, 8, 31
D = 2 * W + 1            # 63
P2 = 2 * D               # 126 partitions = head pair
NPAIR = H // 2           # 4
HD = H * D               # 504
E4 = 4 * E
NCORES = 8
TLOC = (B * T) // NCORES  # 256
NT = TLOC // 128          # 2
FDA = NPAIR * TLOC        # 1024: attention tile free size
NPOLY = 6                 # exp() Taylor degree (rel err ~3e-3, bf16-dominated)
PI = float(np.pi)
TWO_PI = float(2 * np.pi)
EPS = 1e-5


def emit(nc, tc, io, ctx, knobs):
    iters = knobs.get("iters", 0)
    upto = knobs.get("upto", "full")
    if iters:
        ctx.enter_context(tc.For_i(0, iters, 1))
    consts = ctx.enter_context(tc.tile_pool(name="consts", bufs=1))
    acts = ctx.enter_context(tc.tile_pool(name="acts", bufs=1))
    wq = ctx.enter_context(tc.tile_pool(name="wq", bufs=3))
    wf = ctx.enter_context(tc.tile_pool(name="wf", bufs=1))
    wc = ctx.enter_context(tc.tile_pool(name="wc", bufs=4))
    m1p = ctx.enter_context(tc.tile_pool(name="m1p", bufs=1))
    tmp = ctx.enter_context(tc.tile_pool(name="tmp", bufs=2))
    tmps = ctx.enter_context(tc.tile_pool(name="tmps", bufs=3))
    ghp = ctx.enter_context(tc.tile_pool(name="ghp", bufs=3))
    # PSUM: psA/psB one bank x2 bufs, psG two banks x2 bufs = 8 banks exactly.
    psA = ctx.enter_context(tc.tile_pool(name="psA", bufs=2, space="PSUM"))
    psB = ctx.enter_context(tc.tile_pool(name="psB", bufs=2, space="PSUM"))
    psG = ctx.enter_context(tc.tile_pool(name="psG", bufs=2, space="PSUM"))

    # ---------------- input x first (everything waits on it) ----------------
    xtiles = []
    for m in range(NT):
        xtile = tmp.tile([128, 512], F32, tag=f"xin{m}", name=f"xin{m}", bufs=1)
        for hh in range(2):
            nc.sync.dma_start(
                xtile[:, hh * 256:(hh + 1) * 256],
                io["x"].rearrange("(n p) f -> n p f", p=128)[m, :, hh * 256:(hh + 1) * 256])
        xtiles.append(xtile)

    # ---------------- constants (batched DMAs) ----------------
    ident = consts.tile([128, 128], F32, name='ident')
    make_identity(nc, ident[:])

    # per-partition vectors, one [128, 52] tile: invfreq(4) projb(8) fcb(32) cprojb(8)
    cvec = consts.tile([128, 52], F32, name='cvec')
    nc.sync.dma_start(cvec[:], io["cvec"])
    invfreq_t = [cvec[:, i:i + 1] for i in range(0, 4)]
    projb_t = [cvec[:, 4 + i:5 + i] for i in range(8)]
    fcb_t = [cvec[:, 12 + i:13 + i] for i in range(32)]
    cprojb_t = [cvec[:, 44 + i:45 + i] for i in range(8)]

    # row vectors, one [1, 4E] tile: ln1w ln1b ln2w ln2b
    crow = consts.tile([1, 4 * E], F32R, name='crow')
    nc.sync.dma_start(crow[:], io["crow"].rearrange("(o f) -> o f", o=1))
    ln1w_r = [crow[:, 0 * E + i * 128:0 * E + (i + 1) * 128] for i in range(8)]
    ln1b_r = [crow[:, 1 * E + i * 128:1 * E + (i + 1) * 128] for i in range(8)]
    ln2w_r = [crow[:, 2 * E + i * 128:2 * E + (i + 1) * 128] for i in range(8)]
    ln2b_r = [crow[:, 3 * E + i * 128:3 * E + (i + 1) * 128] for i in range(8)]

    ebgh = consts.tile([P2, 2 * (NPOLY + 1) * P2], BF16, name='ebgh')
    nc.sync.dma_start(ebgh[:], io["ebgh"])
    NEB = (NPOLY + 1) * P2
    ebg = ebgh[:, :NEB]
    ebh = ebgh[:, NEB:]

    qkvb_row = consts.tile([1, 3 * HD], F32R, name='qkvb_row')
    nc.sync.dma_start(qkvb_row[:], io["qkvb_pk"].rearrange("(o f) -> o f", o=1))

    def sconst(val, name):
        t = consts.tile([128, 1], F32, tag=name)
        nc.vector.memset(t[:], float(val))
        return t

    c_pi = sconst(PI, "c_pi")
    c_negpi = sconst(-PI, "c_negpi")
    c_halfpi = sconst(PI / 2, "c_halfpi")
    c_neg3halfpi = sconst(-1.5 * PI, "c_neg3halfpi")
    c_n2pi = sconst(-TWO_PI, "c_n2pi")
    c_p2pi = sconst(TWO_PI, "c_p2pi")
    c_eps = sconst(EPS, "c_eps")
    ones_colf = sconst(1.0, "ones_colf")
    ones_col = consts.tile([128, 1], F32R, tag="ones_col", name="ones_col")
    nc.scalar.copy(ones_col[:], ones_colf[:])
    ones_256f = consts.tile([1, TLOC], F32, tag="ones_256f", name="ones_256f")
    nc.vector.memset(ones_256f[:], 1.0)
    ones_256 = consts.tile([1, TLOC], F32R, tag="ones_256", name="ones_256")
    nc.scalar.copy(ones_256[:], ones_256f[:])
    ones_phi = consts.tile([P2, FDA], BF16, tag="ones_phi", name="ones_phi")
    nc.vector.memset(ones_phi[:], 1.0)

    if upto == "noop":
        for m in range(NT):
            z = tmp.tile([128, E], F32, tag="znoop", name="znoop")
            nc.vector.memset(z[:], 0.0)
            nc.sync.dma_start(io["y"].rearrange("(n p) f -> n p f", p=128)[m], z[:])
        return

    # ---------------- transpose x ----------------
    xT = [acts.tile([128, TLOC], F32, tag=f"xT{i}", name=f"xT{i}") for i in range(4)]
    for m in range(NT):
        xtile = xtiles[m]
        for i in range(4):
            ps = psA.tile([128, 512], F32, tag="psA", name="psA")
            nc.tensor.transpose(ps[:, :128], xtile[:, i * 128:(i + 1) * 128], ident[:])
            nc.scalar.copy(xT[i][:, m * 128:(m + 1) * 128], ps[:, :128])

    # ---------------- rotary ----------------
    xrT = [acts.tile([128, TLOC], F32R, tag=f"xrT{i}", name=f"xrT{i}") for i in range(8)]
    for i in range(4):
        ang = tmp.tile([128, TLOC], F32, tag="ang", name="ang")
        nc.vector.tensor_scalar(ang[:], xT[i][:], invfreq_t[i][:], None, ALU.mult)
        if knobs.get("raw_sin"):
            nc.scalar.activation(xrT[i][:], ang[:], ACTF.Sin)
            nc.scalar.activation(xrT[4 + i][:], ang[:], ACTF.Sin, bias=c_halfpi[:])
            continue
        m1 = tmp.tile([128, TLOC], F32, tag="m1", name="m1")
        m2 = tmp.tile([128, TLOC], F32, tag="m2", name="m2")
        r = tmp.tile([128, TLOC], F32, tag="r", name="r")
        nc.vector.tensor_scalar(m1[:], ang[:], c_pi[:], None, ALU.is_gt)
        nc.vector.tensor_scalar(m2[:], ang[:], c_negpi[:], None, ALU.is_lt)
        nc.vector.scalar_tensor_tensor(r[:], m1[:], c_n2pi[:], ang[:], ALU.mult, ALU.add)
        nc.vector.scalar_tensor_tensor(r[:], m2[:], c_p2pi[:], r[:], ALU.mult, ALU.add)
        nc.scalar.activation(xrT[i][:], r[:], ACTF.Sin)
        nc.vector.tensor_scalar(m1[:], ang[:], c_halfpi[:], None, ALU.is_gt)
        nc.vector.tensor_scalar(m2[:], ang[:], c_neg3halfpi[:], None, ALU.is_lt)
        nc.vector.scalar_tensor_tensor(r[:], m1[:], c_n2pi[:], ang[:], ALU.mult, ALU.add)
        nc.vector.scalar_tensor_tensor(r[:], m2[:], c_p2pi[:], r[:], ALU.mult, ALU.add)
        nc.scalar.activation(xrT[4 + i][:], r[:], ACTF.Sin, bias=c_halfpi[:])

    def finish_featmajor(tiles8):
        for e in range(8):
            src_t = tiles8[e]
            sap = src_t[:].bitcast(F32) if src_t.dtype in (F32R,) else (
                src_t[:] if src_t.dtype == F32 else src_t[:].bitcast(F32))
            for m in range(NT):
                ps = psA.tile([128, 512], F32, tag="psA", name="psAf")
                nc.tensor.transpose(ps[:, :128], sap[:, m * 128:(m + 1) * 128], ident[:])
                ysb = tmp.tile([128, 128], F32, tag="ysb", name="ysbf")
                nc.scalar.copy(ysb[:], ps[:, :128])
                nc.sync.dma_start(
                    io["y"].rearrange("(n p) f -> n p f", p=128)[m, :, e * 128:(e + 1) * 128],
                    ysb[:])

    if upto == "rotary":
        finish_featmajor(xrT)
        return

    # ---------------- layernorm helper (feat-major over 8 tiles) ----------------
    def layernorm(src_tiles, w_rows, b_rows, out_tag, out_dt=BF16):
        sum_ps = psA.tile([128, 512], F32, tag="psA", name="psA")
        sq_ps = psB.tile([128, 512], F32, tag="psB", name="psB")
        for i in range(8):
            nc.tensor.matmul(sum_ps[:1, :TLOC], ones_col[:], src_tiles[i][:],
                             start=(i == 0), stop=(i == 7))
        for i in range(8):
            sq = tmp.tile([128, TLOC], F32R, tag="lnsq", name="lnsq")
            nc.scalar.activation(sq[:], src_tiles[i][:].bitcast(F32), ACTF.Square)
            nc.tensor.matmul(sq_ps[:1, :TLOC], ones_col[:], sq[:],
                             start=(i == 0), stop=(i == 7))
        row = tmps.tile([1, 4 * TLOC], F32R, tag="lnrow", name="lnrow", bufs=1)
        mu = row[:, 0:TLOC]
        var = row[:, TLOC:2 * TLOC]
        rstd = row[:, 2 * TLOC:3 * TLOC]
        nrm = row[:, 3 * TLOC:4 * TLOC]
        _f = lambda ap: ap.bitcast(F32)
        nc.scalar.mul(mu, sum_ps[:1, :TLOC], 1.0 / E)
        nc.vector.tensor_tensor(nrm, _f(mu), _f(mu), ALU.mult)  # nrm as musq scratch
        nc.vector.scalar_tensor_tensor(var, sq_ps[:1, :TLOC], 1.0 / E, _f(nrm),
                                       ALU.mult, ALU.subtract)
        nc.vector.tensor_scalar(var, _f(var), c_eps[:1, :], None, ALU.add)
        nc.scalar.activation(var, _f(var), ACTF.Ln)
        nc.scalar.activation(rstd, _f(var), ACTF.Exp, scale=-0.5)
        nc.vector.tensor_tensor(nrm, _f(mu), _f(rstd), ALU.mult)
        nc.scalar.mul(nrm, _f(nrm), -1.0)
        outs = []
        for i in range(8):
            a_ps = psA.tile([128, 512], F32, tag="psA", name="psA")
            b_ps = psB.tile([128, 512], F32, tag="psB", name="psB")
            nc.tensor.matmul(a_ps[:, :TLOC], w_rows[i][:], rstd,
                             start=True, stop=True)
            nc.tensor.matmul(b_ps[:, :TLOC], w_rows[i][:], nrm,
                             start=True, stop=False)
            nc.tensor.matmul(b_ps[:, :TLOC], b_rows[i][:], ones_256[:],
                             start=False, stop=True)
            o = acts.tile([128, TLOC], out_dt, tag=f"{out_tag}{i}", name=f"{out_tag}{i}")
            t1 = tmp.tile([128, TLOC], F32, tag="lnt1", name="lnt1")
            nc.vector.tensor_tensor(t1[:], src_tiles[i][:].bitcast(F32),
                                    a_ps[:, :TLOC], ALU.mult)
            nc.vector.tensor_tensor(o[:], t1[:], b_ps[:, :TLOC], ALU.add)
            outs.append(o)
        return outs

    hT = layernorm(xrT, ln1w_r, ln1b_r, "lnout", out_dt=BF16)
    if upto == "ln1":
        finish_featmajor(hT)
        return

    # ---------------- qkv (feature-major, head-pair-packed out) ----------------
    # qkvf[c] layout: [126 part = (parity, d), 4 pairs x 256 tokens]
    qkvf = [acts.tile([P2, FDA], BF16, tag=f"qkvf{c}", name=f"qkvf{c}")
            for c in range(3)]
    qkvw_src = io["qkvw_pk"].rearrange("(n p) f -> n p f", p=128)
    wts = []
    for k in range(8):
        wt = wq.tile([128, 3 * HD], BF16, tag=f"qkvw{k}", name=f"qkvw{k}", bufs=1)
        nc.sync.dma_start(wt[:], qkvw_src[k])
        wts.append(wt)
    for c in range(3):
        for j in range(NPAIR):
            col0 = c * HD + j * P2
            ps = psA.tile([128, 512], F32, tag="psA", name="psA")
            for k in range(8):
                nc.tensor.matmul(ps[:P2, :TLOC], wts[k][:, col0:col0 + P2],
                                 hT[k][:], start=(k == 0), stop=False)
            nc.tensor.matmul(ps[:P2, :TLOC], qkvb_row[:, col0:col0 + P2],
                             ones_256[:], start=False, stop=True)
            if (c * NPAIR + j) % 2 == 0:
                nc.scalar.copy(qkvf[c][:, j * TLOC:(j + 1) * TLOC], ps[:P2, :TLOC])
            else:
                nc.vector.tensor_copy(qkvf[c][:, j * TLOC:(j + 1) * TLOC],
                                      ps[:P2, :TLOC])
    qf, kf, vf = qkvf

    # ---------------- attention (polynomial exp, PE contractions) ----------------
    # phi[n] = q^n (bf16), n = 0..NPOLY
    phi = [ones_phi, qf]
    for n in range(2, NPOLY + 1):
        p = acts.tile([P2, FDA], BF16, tag=f"phi{n}", name=f"phi{n}")
        nc.vector.tensor_tensor(p[:], phi[n - 1][:], qf[:], ALU.mult)
        phi.append(p)

    def eb_matmul(weights, n, rhs_tile):
        gp = psG.tile([128, FDA], F32, tag="psG", name="psG")
        lhs = weights[:, n * P2:(n + 1) * P2]
        for hh in range(FDA // 512):
            nc.tensor.matmul(gp[:P2, hh * 512:(hh + 1) * 512], lhs,
                             rhs_tile[:, hh * 512:(hh + 1) * 512],
                             start=True, stop=True)
        return gp

    # g = sum_n k^n * ((EB/n!)^T @ q^n), Horner descending in n
    acc_g = acts.tile([P2, FDA], BF16, tag="acc_g", name="acc_g")
    for n in range(NPOLY, -1, -1):
        gp = eb_matmul(ebg, n, phi[n])
        if n == NPOLY:
            nc.scalar.copy(acc_g[:], gp[:P2, :])
        else:
            gs = ghp.tile([P2, FDA], BF16, tag="gsb", name="gsb")
            nc.scalar.copy(gs[:], gp[:P2, :])
            nc.vector.tensor_tensor(acc_g[:], acc_g[:], kf[:], ALU.mult)
            nc.vector.tensor_tensor(acc_g[:], acc_g[:], gs[:], ALU.add)

    # u = v / g
    u = acts.tile([P2, FDA], BF16, tag="u_t", name="u_t")
    with nc.allow_low_precision("bf16 attention denominator"):
        nc.vector.reciprocal(u[:], acc_g[:])
    nc.vector.tensor_tensor(u[:], u[:], vf[:], ALU.mult)

    # out = sum_n q^n * ((EB/n!) @ (k^n * u)), ascending accumulation.
    # The zt chain (k^n * u) runs 2 levels ahead of the phh/add consumers so
    # the DVE never stalls on the PE->ACT copy round-trip of H_n.
    out_acc = acts.tile([P2, FDA], BF16, tag="out_acc", name="out_acc")
    zts = [u]
    hss = [None] * (NPOLY + 1)
    LAG = 2

    def emit_produce(n):
        if n > NPOLY:
            return
        if n >= 1:
            zt = ghp.tile([P2, FDA], BF16, tag="zt", name=f"zt{n}", bufs=LAG + 2)
            nc.vector.tensor_tensor(zt[:], zts[n - 1][:], kf[:], ALU.mult)
            zts.append(zt)
        hp = eb_matmul(ebh, n, zts[n])
        if n == 0:
            nc.scalar.copy(out_acc[:], hp[:P2, :])
        else:
            hs = ghp.tile([P2, FDA], BF16, tag="gsb", name="hsb")
            nc.scalar.copy(hs[:], hp[:P2, :])
            hss[n] = hs

    def emit_consume(n):
        if not (1 <= n <= NPOLY):
            return
        tt = tmps.tile([P2, FDA], BF16, tag="phh", name="phh")
        nc.vector.tensor_tensor(tt[:], phi[n][:], hss[n][:], ALU.mult)
        nc.vector.tensor_tensor(out_acc[:], out_acc[:], tt[:], ALU.add)

    for n in range(0, NPOLY + 1 + LAG):
        emit_produce(n)
        emit_consume(n - LAG)

    # ---------------- proj + residual ----------------
    pw = []
    pw_src = io["pw_pk"].rearrange("(j p) f -> j p f", p=P2)
    for j in range(NPAIR):
        wt = wq.tile([P2, E], BF16, tag=f"pw{j}", name=f"pw{j}", bufs=1)
        nc.sync.dma_start(wt[:], pw_src[j])
        pw.append(wt)
    xaT = []
    for i in range(8):
        ps = psA.tile([128, 512], F32, tag="psA", name="psA")
        for j in range(NPAIR):
            nc.tensor.matmul(ps[:, :TLOC], pw[j][:, i * 128:(i + 1) * 128],
                             out_acc[:, j * TLOC:(j + 1) * TLOC],
                             start=(j == 0), stop=(j == 3))
        o = acts.tile([128, TLOC], F32R, tag=f"xaT{i}", name=f"xaT{i}")
        nc.vector.scalar_tensor_tensor(o[:], ps[:, :TLOC], projb_t[i][:],
                                       xrT[i][:].bitcast(F32), ALU.add, ALU.add)
        xaT.append(o)

    if upto == "proj":
        finish_featmajor(xaT)
        return

    # ---------------- LN2 ----------------
    h2T = layernorm(xaT, ln2w_r, ln2b_r, "lnout", out_dt=BF16)  # reuses lnout slots

    # ---------------- fc + gelu -> m1g (bf16), then cproj ----------------
    m1g = [m1p.tile([128, TLOC], BF16, tag=f"m1g{j}", name=f"m1g{j}") for j in range(32)]
    fw_g = io["fw_t"].rearrange("(k p) (g f) -> p k g f", p=128, f=256)  # [128,8,16,256]
    for jg in range(16):          # groups of 2 j-tiles
        fwg = wf.tile([128, 8, 256], BF16, tag="fwg", name="fwg", bufs=4)
        nc.sync.dma_start(fwg[:], fw_g[:, :, jg, :])
        for jj in range(2):
            j = jg * 2 + jj
            fps = psB.tile([128, 512], F32, tag="psB", name="psB")
            for k in range(8):
                nc.tensor.matmul(fps[:, :TLOC], fwg[:, k, jj * 128:(jj + 1) * 128],
                                 h2T[k][:], start=(k == 0), stop=(k == 7))
            gelu_f = ACTF.Tanh if knobs.get("sim_tanh") else ACTF.Gelu
            nc.scalar.activation(m1g[j][:], fps[:, :TLOC], gelu_f, bias=fcb_t[j][:])
    # cproj: e-outer, contract over 32 j-tiles
    cw_src = io["cw_te"].rearrange("(e p) f -> e p f", p=128)    # [8,128,4096]
    for e in range(8):
        cps = psG.tile([128, FDA], F32, tag="psG", name="cpp")
        for half in range(2):
            cwt = wc.tile([128, E4 // 2], BF16, tag="cwt", name="cwt")
            nc.sync.dma_start(cwt[:], cw_src[e, :, half * 2048:(half + 1) * 2048])
            for jj in range(16):
                j = half * 16 + jj
                nc.tensor.matmul(cps[:, :TLOC], cwt[:, jj * 128:(jj + 1) * 128],
                                 m1g[j][:], start=(j == 0), stop=(j == 31))
        yT = tmp.tile([128, TLOC], F32, tag="yT", name="yT")
        nc.vector.scalar_tensor_tensor(yT[:], cps[:, :TLOC], cprojb_t[e][:],
                                       xaT[e][:].bitcast(F32), ALU.add, ALU.add)
        for m in range(NT):
            ps = psB.tile([128, 512], F32, tag="psB", name="psB")
            nc.tensor.transpose(ps[:, :128], yT[:, m * 128:(m + 1) * 128], ident[:])
            ysb = tmp.tile([128, 128], F32, tag="ysb", name="ysb")
            nc.scalar.copy(ysb[:], ps[:, :128])
            nc.sync.dma_start(
                io["y"].rearrange("(n p) f -> n p f", p=128)[m, :, e * 128:(e + 1) * 128],
                ysb[:])


def build(knobs=None):
    from contextlib import ExitStack
    knobs = knobs or {}
    nc = bacc.Bacc("TRN2", target_bir_lowering=False, debug=False)
    io = {}

    def din(name, shape, dt=F32):
        io[name] = nc.dram_tensor(name, shape, dt, kind="ExternalInput").ap()

    din("x", [TLOC, 512])
    din("qkvw_pk", [E, 3 * HD], BF16)
    din("qkvb_pk", [3 * HD], F32R)
    din("ebgh", [P2, 2 * (NPOLY + 1) * P2], BF16)
    din("pw_pk", [HD, E], BF16)
    din("fw_t", [E, E4], BF16)
    din("cw_te", [E, E4], BF16)     # per-e k-major packing, see host_prep
    din("cvec", [128, 52])          # invfreq | projb | fcb | cprojb columns
    din("crow", [4 * E], F32R)      # ln1w | ln1b | ln2w | ln2b
    io["y"] = nc.dram_tensor("y", [TLOC, E], F32, kind="ExternalOutput").ap()

    with tile.TileContext(nc) as tc:
        with ExitStack() as ctx:
            emit(nc, tc, io, ctx, knobs)
    nc.compile()
    return nc


def host_prep(inputs):
    x = np.asarray(inputs["x"], np.float32).reshape(B * T, E // 2)
    qkv_w = np.asarray(inputs["qkv_w"], np.float32)
    qkv_b = np.asarray(inputs["qkv_b"], np.float32)
    rel_pos = np.asarray(inputs["rel_pos"], np.float32)
    proj_w = np.asarray(inputs["proj_w"], np.float32)
    fc_w = np.asarray(inputs["fc_w"], np.float32)
    cproj_w = np.asarray(inputs["cproj_w"], np.float32)

    inv_freq = (1.0 / 10000.0 ** (np.arange(0, E, 2, dtype=np.float32) / E)).astype(np.float32)

    # head-pair packing permutation: new (c, j, parity, d) <- old (c, h=2j+parity, d)
    colperm = np.empty(3 * HD, np.int64)
    for c in range(3):
        for j in range(NPAIR):
            for par in range(2):
                h = 2 * j + par
                dst = c * HD + j * P2 + par * D
                src = c * HD + h * D
                colperm[dst:dst + D] = np.arange(src, src + D)
    qkvw_pk = np.ascontiguousarray(qkv_w.T[:, colperm].astype(ml_dtypes.bfloat16))
    qkvb_pk = np.ascontiguousarray(qkv_b[colperm])

    perm = np.arange(-W, W + 1) % D
    EB = np.exp(rel_pos[perm]).astype(np.float64)        # [d, v]
    EBbd = np.zeros((P2, P2))
    EBbd[:D, :D] = EB
    EBbd[D:, D:] = EB
    ebg = np.concatenate(
        [EBbd / math.factorial(n) for n in range(NPOLY + 1)], axis=1)
    ebh = np.concatenate(
        [EBbd.T / math.factorial(n) for n in range(NPOLY + 1)], axis=1)

    rowperm = colperm[:HD]    # same (j, parity, d) <- (h, d) reorder
    pw_pk = np.ascontiguousarray(proj_w.T[rowperm].astype(ml_dtypes.bfloat16))

    # cw_te[e]: [4096, 128] column-block e of cproj_w.T, repacked so SBUF tile
    # [128, 4096] holds k-tile j at cols j*128:(j+1)*128
    cw_t = cproj_w.T.astype(ml_dtypes.bfloat16)          # [4096, 1024]
    cw_te = np.empty((E, E4), ml_dtypes.bfloat16)
    for e in range(8):
        blk = cw_t[:, e * 128:(e + 1) * 128]             # [4096, 128]
        cw_te[e * 128:(e + 1) * 128] = (
            blk.reshape(32, 128, 128).transpose(1, 0, 2).reshape(128, E4))

    cvec = np.zeros((128, 52), np.float32)
    cvec[:, 0:4] = inv_freq.reshape(4, 128).T
    cvec[:, 4:12] = np.asarray(inputs["proj_b"], np.float32).reshape(8, 128).T
    cvec[:, 12:44] = np.asarray(inputs["fc_b"], np.float32).reshape(32, 128).T
    cvec[:, 44:52] = np.asarray(inputs["cproj_b"], np.float32).reshape(8, 128).T
    crow = np.concatenate([
        np.asarray(inputs["ln1_w"], np.float32),
        np.asarray(inputs["ln1_b"], np.float32),
        np.asarray(inputs["ln2_w"], np.float32),
        np.asarray(inputs["ln2_b"], np.float32)])

    common = {
        "qkvw_pk": qkvw_pk,
        "qkvb_pk": qkvb_pk,
        "ebgh": np.concatenate([ebg, ebh], axis=1).astype(ml_dtypes.bfloat16),
        "pw_pk": pw_pk,
        "fw_t": np.ascontiguousarray(fc_w.T.astype(ml_dtypes.bfloat16)),
        "cw_te": cw_te,
        "cvec": cvec,
        "crow": crow,
    }
    in_maps = []
    for c in range(NCORES):
        m = dict(common)
        m["x"] = np.ascontiguousarray(x[c * TLOC:(c + 1) * TLOC])
        in_maps.append(m)
    return in_maps


def kernel(**inputs):
    nc = build()
    in_maps = host_prep(inputs)
    res = run_bass_kernel_spmd(nc, in_maps, list(range(NCORES))).results
    y = np.concatenate([res[c]["y"] for c in range(NCORES)], axis=0)
    return y.reshape(B, T, E)
